# revision 4
# baseline (speedup 1.0000x reference)
"""MemTransformerLM (Transformer-XL) forward pass on 8 TRN2 NeuronCores.

Sharding: core c handles batch b = c//2 and tensor-parallel half h = c%2
(heads 8h..8h+8 of 16; FFN inner columns 2048h..2048h+2048 of 4096; vocab
16000h.. for the final logsumexp). Pairwise AllGather + local add after the
attention output projection and after FFN2.

Layout: the residual stream lives TRANSPOSED in SBUF as x[p, dc, i] =
x[i, 128*dc+p] (d on partitions), bf16. All projections consume it directly
as the matmul moving operand; attention scores are computed transposed
(scoreT[j, i], klen on partitions) so softmax probs feed PV without any
transpose. Softmax is unnormalized: exp(score*scale) accumulates through PV
and the out-projection input is scaled by 1/den per head beforehand.

rel_shift: pre[i, jj] = (q+br)_i . rk_jj is written to DRAM with SHINGLED
rows (row i at element offset i*1025 + 1). Then the plain dense [512, 1024]
view at element offset 512 satisfies dense[i, j] = pre[i, j + 511 - i] =
shifted BD, and a transpose-DMA of that view yields BD^T[j, i] directly.
Positions j > i + 512 read garbage; affine_select fills them with -1e30
(the causal mask), so exp gives exactly 0.

LayerNorm in transposed layout: token sums via ones-matmul into PSUM[1, i],
broadcast back across partitions, elementwise apply with per-partition g/b.

All matmuls bf16 with fp32 PSUM accumulation.
"""

import numpy as np
import ml_dtypes

import concourse.bass as bass
import concourse.mybir as mybir
import concourse.tile as tile
from concourse import bacc
from concourse.bass_utils import run_bass_kernel_spmd

# Model dims (hardcoded per problem spec)
L = 6
D_MODEL = 1024
D_HEAD = 64
D_INNER = 4096
BSZ = 4
QLEN = 512
MLEN = 512
KLEN = MLEN + QLEN
VOCAB = 32000
SCALE = 1.0 / (D_HEAD ** 0.5)
EPS = 1e-5
NEG = -1e30

NCORES = 8
NDH = 512          # nd per core (8 heads x 64)
DIH = 2048         # ffn inner per core
VSH = VOCAB // 2   # vocab per core (split across the pair)
VT = 400           # vocab tile width
NVT = VSH // VT    # 40

# shingled scratch: row i of pre written at element offset 1 + i*1025
SCR_N = 512 * 1025 + 1024 + 64

DT = mybir.dt.float32
BF = mybir.dt.bfloat16
F32 = np.float32
BF16 = ml_dtypes.bfloat16

PAIRS = [[0, 1], [2, 3], [4, 5], [6, 7]]

_CACHE: dict = {}


def _build():
    nc = bacc.Bacc("TRN2", target_bir_lowering=False, debug=False, num_devices=NCORES)

    # ---- I/O ----
    x0_in = nc.dram_tensor("x0", [128, 8, QLEN], BF, kind="ExternalInput")
    memT_in = nc.dram_tensor("memT", [L, 128, 8, MLEN], BF, kind="ExternalInput")
    wq_in = nc.dram_tensor("wq", [L, 128, 8, NDH], BF, kind="ExternalInput")
    wk_in = nc.dram_tensor("wk", [L, 128, 8, NDH], BF, kind="ExternalInput")
    wv_in = nc.dram_tensor("wv", [L, 128, 8, NDH], BF, kind="ExternalInput")
    rkT_in = nc.dram_tensor("rkT", [L, 4, 128, KLEN], BF, kind="ExternalInput")
    wo_in = nc.dram_tensor("wo", [L, 128, 4, D_MODEL], BF, kind="ExternalInput")
    # w1 regrouped per inner-chunk: [L, 16, 128, 8, 128]
    w1_in = nc.dram_tensor("w1", [L, 16, 128, 8, 128], BF, kind="ExternalInput")
    # w2 regrouped per dmodel-chunk: [L, 8, 128, 16, 128]
    w2_in = nc.dram_tensor("w2", [L, 8, 128, 16, 128], BF, kind="ExternalInput")
    b1_in = nc.dram_tensor("b1", [L, 128, 16], DT, kind="ExternalInput")
    # packed per-partition params: [g1, bg1, g2, bg2, b2] along dim 2
    lns_in = nc.dram_tensor("lns", [L, 128, 5, 8], DT, kind="ExternalInput")
    bw_in = nc.dram_tensor("bw", [128, 4], DT, kind="ExternalInput")
    br_in = nc.dram_tensor("br", [128, 4], DT, kind="ExternalInput")
    embT_in = nc.dram_tensor("embT", [NVT, 128, 8, VT], BF, kind="ExternalInput")

    xout = nc.dram_tensor("xout", [128, 8, QLEN], BF, kind="ExternalOutput")
    lmax_out = nc.dram_tensor("lmax", [128, 4, NVT], DT, kind="ExternalOutput")
    lsum_out = nc.dram_tensor("lsum", [128, 4, NVT], DT, kind="ExternalOutput")

    with tile.TileContext(nc) as tc:
        with (
            tc.tile_pool(name="const", bufs=1) as constp,
            tc.tile_pool(name="res", bufs=1) as resp,
            tc.tile_pool(name="wts", bufs=1) as wtp,
            tc.tile_pool(name="wst", bufs=3) as wstp,     # streamed weight tiles
            tc.tile_pool(name="act", bufs=1) as actp,
            tc.tile_pool(name="prq", bufs=2) as prqp,     # per-pair q/k/rk tiles
            tc.tile_pool(name="hd", bufs=2) as hdp,       # per-head bd/exp tiles
            tc.tile_pool(name="pre2", bufs=2) as prep,    # pre SBUF staging
            tc.tile_pool(name="lnp", bufs=2) as lnp,      # LN scratch
            tc.tile_pool(name="small", bufs=4) as smp,
            tc.tile_pool(name="arp", bufs=2) as arp,      # collective readback
            tc.tile_pool(name="ps_ac", bufs=3, space="PSUM") as psA,
            tc.tile_pool(name="ps_pre", bufs=2, space="PSUM") as psSp,
            tc.tile_pool(name="ps_pv", bufs=1, space="PSUM") as psV,
            tc.tile_pool(name="ps_proj", bufs=2, space="PSUM") as psP,
            tc.tile_pool(name="dram", bufs=2, space="DRAM") as dramp,
        ):
            bw_t = constp.tile([128, 4], DT)
            br_t = constp.tile([128, 4], DT)
            ones_t = constp.tile([128, 1], BF)
            nc.sync.dma_start(bw_t[:], bw_in[:])
            nc.sync.dma_start(br_t[:], br_in[:])
            nc.vector.memset(ones_t[:], 1.0)

            # residual stream, bf16, transposed: x[p, dc, i] = x[i, 128 dc + p]
            x_bf = resp.tile([128, 8, QLEN], BF)
            nc.sync.dma_start(x_bf[:], x0_in[:])
            lmax_sb = resp.tile([128, 4, NVT], DT)
            lsum_sb = resp.tile([128, 4, NVT], DT)

            def layer_norm(g_t, b_t):
                """LN over d (partitions x 8 chunks) of x_bf, in place."""
                sq = lnp.tile([128, QLEN], BF, tag="lnsq")
                xs_ps = psP.tile([1, QLEN], DT, tag="proj")
                sq_ps = psP.tile([1, QLEN], DT, tag="proj")
                for dc in range(8):
                    nc.vector.tensor_tensor(
                        sq[:], x_bf[:, dc, :], x_bf[:, dc, :], mybir.AluOpType.mult
                    )
                    nc.tensor.matmul(
                        xs_ps[:], ones_t[:], x_bf[:, dc, :],
                        start=(dc == 0), stop=(dc == 7),
                    )
                    nc.tensor.matmul(
                        sq_ps[:], ones_t[:], sq[:],
                        start=(dc == 0), stop=(dc == 7),
                    )
                xs_sb = smp.tile([1, QLEN], DT, tag="xs")
                sq_sb = smp.tile([1, QLEN], DT, tag="sqs")
                nc.vector.tensor_copy(xs_sb[:], xs_ps[:])
                nc.vector.tensor_copy(sq_sb[:], sq_ps[:])
                XS = lnp.tile([128, QLEN], DT, tag="XS")
                SQ = lnp.tile([128, QLEN], DT, tag="SQ")
                nc.gpsimd.partition_broadcast(XS[:], xs_sb[:])
                nc.gpsimd.partition_broadcast(SQ[:], sq_sb[:])
                mu = lnp.tile([128, QLEN], DT, tag="mu")
                var = lnp.tile([128, QLEN], DT, tag="var")
                rstd = lnp.tile([128, QLEN], DT, tag="rstd")
                nc.vector.tensor_scalar_mul(mu[:], XS[:], 1.0 / D_MODEL)
                nc.vector.tensor_tensor(var[:], mu[:], mu[:], mybir.AluOpType.mult)
                nc.vector.tensor_scalar_mul(SQ[:], SQ[:], 1.0 / D_MODEL)
                nc.vector.tensor_tensor(
                    var[:], SQ[:], var[:], mybir.AluOpType.subtract
                )
                nc.scalar.activation(
                    rstd[:], var[:], mybir.ActivationFunctionType.Rsqrt, bias=EPS
                )
                for dc in range(8):
                    xc = lnp.tile([128, QLEN], DT, tag="xc")
                    nc.vector.tensor_tensor(
                        xc[:], x_bf[:, dc, :], mu[:], mybir.AluOpType.subtract
                    )
                    nc.vector.tensor_tensor(
                        xc[:], xc[:], rstd[:], mybir.AluOpType.mult
                    )
                    nc.vector.tensor_scalar(
                        x_bf[:, dc, :], xc[:], g_t[:, dc : dc + 1],
                        b_t[:, dc : dc + 1],
                        mybir.AluOpType.mult, mybir.AluOpType.add,
                    )

            def add_residual(ar_out, b2_t=None):
                """x_bf += allgathered partial sums (+ b2)."""
                arr4 = ar_out.rearrange("r (c p) i -> r p c i", p=128)
                for dc in range(8):
                    arr = arp.tile([128, QLEN], BF, tag="arr")
                    nc.sync.dma_start(arr[:], arr4[0, :, dc, :])
                    nc.gpsimd.dma_start(
                        arr[:], arr4[1, :, dc, :], accum_op=mybir.AluOpType.add
                    )
                    if b2_t is not None:
                        nc.vector.tensor_scalar(
                            arr[:], arr[:], b2_t[:, dc : dc + 1], None,
                            mybir.AluOpType.add,
                        )
                    nc.vector.tensor_tensor(
                        x_bf[:, dc, :], x_bf[:, dc, :], arr[:], mybir.AluOpType.add
                    )

            for l in range(L):
                # ---- whole-layer loads ----
                wq_t = wtp.tile([128, 8, NDH], BF, tag="wq")
                wk_t = wtp.tile([128, 8, NDH], BF, tag="wk")
                wv_t = wtp.tile([128, 8, NDH], BF, tag="wv")
                nc.sync.dma_start(wq_t[:], wq_in[l])
                nc.sync.dma_start(wk_t[:], wk_in[l])
                nc.sync.dma_start(wv_t[:], wv_in[l])
                memT_t = actp.tile([128, 8, MLEN], BF, tag="memT")
                nc.sync.dma_start(memT_t[:], memT_in[l])
                b1_t = wtp.tile([128, 16], DT, tag="b1")
                nc.sync.dma_start(b1_t[:], b1_in[l])
                lnt = wtp.tile([128, 8, 5], DT, tag="lns")
                for i, src in enumerate((g1_in, bg1_in, g2_in, bg2_in, b2_in)):
                    nc.gpsimd.dma_start(lnt[:, :, i], src[l])

                # ---- v projection: vv[p, kc, n] = v[128 kc + p, n], all heads ----
                vv = actp.tile([128, 8, NDH], BF, tag="vv")
                for kc in range(8):
                    vps = psP.tile([128, NDH], DT, tag="proj")
                    src = memT_t if kc < 4 else x_bf
                    ksl = slice(128 * (kc % 4), 128 * (kc % 4) + 128)
                    for dc in range(8):
                        nc.tensor.matmul(
                            vps[:], src[:, dc, ksl], wv_t[:, dc, :],
                            start=(dc == 0), stop=(dc == 7),
                        )
                    nc.vector.tensor_copy(vv[:, kc, :], vps[:])

                pvT_all = actp.tile([128, 4, QLEN], BF, tag="pvT")

                # ---- attention, one 2-head pair (= one 128-row nd chunk) at a time
                for p in range(4):
                    nsl = slice(128 * p, 128 * p + 128)
                    # q^T + biases
                    qps = psP.tile([128, QLEN], DT, tag="proj")
                    for dc in range(8):
                        nc.tensor.matmul(
                            qps[:], wq_t[:, dc, nsl], x_bf[:, dc, :],
                            start=(dc == 0), stop=(dc == 7),
                        )
                    qbwT = prqp.tile([128, QLEN], BF, tag="qbw")
                    qbrT = prqp.tile([128, QLEN], BF, tag="qbr")
                    nc.scalar.add(qbwT[:], qps[:], bw_t[:, p : p + 1])
                    nc.scalar.add(qbrT[:], qps[:], br_t[:, p : p + 1])
                    # k^T chunk: kT[p2, j] with p2 = nd within chunk
                    kT = prqp.tile([128, KLEN], BF, tag="kT")
                    for kh in range(2):
                        kps = psP.tile([128, QLEN], DT, tag="proj")
                        src = memT_t if kh == 0 else x_bf
                        for dc in range(8):
                            nc.tensor.matmul(
                                kps[:], wk_t[:, dc, nsl], src[:, dc, :],
                                start=(dc == 0), stop=(dc == 7),
                            )
                        nc.vector.tensor_copy(kT[:, 512 * kh : 512 * kh + 512], kps[:])
                    rkT = prqp.tile([128, KLEN], BF, tag="rkT")
                    nc.sync.dma_start(rkT[:], rkT_in[l, p])

                    # pre = (q+br)^T-chunk @ rkT per head, shingled-write to DRAM
                    scrs = []
                    for hh in range(2):
                        scr = dramp.tile([SCR_N], BF, tag=f"scr{hh}")
                        scrs.append(scr)
                    for ic in range(4):
                        pre_sb = prep.tile([128, 2, KLEN], BF, tag="pre_sb")
                        for hh in range(2):
                            base = 64 * hh
                            for jh in range(2):
                                pps = psSp.tile([128, 512], DT, tag="pre")
                                nc.tensor.matmul(
                                    pps[:],
                                    qbrT[base : base + 64, 128 * ic : 128 * ic + 128],
                                    rkT[base : base + 64, 512 * jh : 512 * jh + 512],
                                    start=True, stop=True,
                                )
                                nc.vector.tensor_copy(
                                    pre_sb[:, hh, 512 * jh : 512 * jh + 512], pps[:]
                                )
                        for hh in range(2):
                            shingle = bass.AP(
                                scrs[hh].tensor,
                                scrs[hh].offset + 1 + 128 * ic * 1025,
                                [[1025, 128], [1, KLEN]],
                            )
                            nc.sync.dma_start(shingle, pre_sb[:, hh, :])

                    # per head: transpose-read shifted BD^T, mask, AC, exp, PV
                    for hh in range(2):
                        base = 64 * hh
                        h2 = 2 * p + hh
                        scr = scrs[hh]
                        dense = scr[512 : 512 + QLEN * KLEN].rearrange(
                            "(i j) -> i j", j=KLEN
                        )
                        bd = hdp.tile([128, 8, QLEN], BF, tag=f"bd{hh}")
                        nc.scalar.dma_start_transpose(bd[:, 0:4, :], dense[:, 0:512])
                        nc.scalar.dma_start_transpose(bd[:, 4:8, :], dense[:, 512:])
                        # mask: keep i >= 128 jc + pj - 512 (only jc>=4 can mask)
                        for jc in range(4, 8):
                            w = 128 * (jc - 3)
                            nc.gpsimd.affine_select(
                                out=bd[:, jc, 0:w], in_=bd[:, jc, 0:w],
                                pattern=[[1, w]],
                                compare_op=mybir.AluOpType.is_ge,
                                fill=NEG, base=512 - 128 * jc,
                                channel_multiplier=-1,
                            )
                        expT = hdp.tile([128, 8, QLEN], BF, tag=f"expT{hh}")
                        for jc in range(8):
                            acp = psA.tile([128, QLEN], DT, tag="ac")
                            nc.tensor.matmul(
                                acp[:],
                                kT[base : base + 64, 128 * jc : 128 * jc + 128],
                                qbwT[base : base + 64, :],
                                start=True, stop=True,
                            )
                            nc.vector.tensor_tensor(
                                acp[:], acp[:], bd[:, jc, :], mybir.AluOpType.add
                            )
                            nc.scalar.activation(
                                expT[:, jc, :], acp[:],
                                mybir.ActivationFunctionType.Exp, scale=SCALE,
                            )
                        # denominator: sum over j = chunk-add then ones-matmul
                        esum = hdp.tile([128, QLEN], BF, tag=f"esum{hh}")
                        nc.vector.tensor_tensor(
                            esum[:], expT[:, 0, :], expT[:, 1, :], mybir.AluOpType.add
                        )
                        for jc in range(2, 8):
                            nc.vector.tensor_tensor(
                                esum[:], esum[:], expT[:, jc, :], mybir.AluOpType.add
                            )
                        den_ps = psSp.tile([1, QLEN], DT, tag="pre")
                        nc.tensor.matmul(
                            den_ps[:], ones_t[:], esum[:], start=True, stop=True
                        )
                        rec = smp.tile([1, QLEN], DT, tag="rec")
                        nc.vector.reciprocal(rec[:], den_ps[:])
                        recb = hdp.tile([64, QLEN], DT, tag=f"recb{hh}")
                        nc.gpsimd.partition_broadcast(recb[:], rec[:])
                        # PV: vecT_unnorm[d, i], 64 rows per head
                        pv = psV.tile([64, QLEN], DT, tag=f"pv{hh}")
                        for jc in range(8):
                            nc.tensor.matmul(
                                pv[:],
                                vv[:, jc, 64 * h2 : 64 * h2 + 64],
                                expT[:, jc, :],
                                start=(jc == 0), stop=(jc == 7),
                            )
                        nc.vector.tensor_tensor(
                            pvT_all[base : base + 64, p, :], pv[:], recb[:],
                            mybir.AluOpType.mult,
                        )

                # ---- attention out projection (transposed) + pairwise exchange
                wo_t = wtp.tile([128, 4, D_MODEL], BF, tag="wq")  # alias wq slot
                nc.sync.dma_start(wo_t[:], wo_in[l])
                ar_in = dramp.tile([D_MODEL, QLEN], BF, tag="arin")
                ar_out = dramp.tile([2, D_MODEL, QLEN], BF, tag="arout")
                ar4 = ar_in.rearrange("(c p) i -> p c i", p=128)
                for dmc in range(8):
                    ops = psP.tile([128, QLEN], DT, tag="proj")
                    for p in range(4):
                        nc.tensor.matmul(
                            ops[:],
                            wo_t[:, p, 128 * dmc : 128 * dmc + 128],
                            pvT_all[:, p, :],
                            start=(p == 0), stop=(p == 3),
                        )
                    osb = arp.tile([128, QLEN], BF, tag="osb")
                    nc.vector.tensor_copy(osb[:], ops[:])
                    nc.sync.dma_start(ar4[:, dmc, :], osb[:])
                nc.gpsimd.collective_compute(
                    "AllGather", mybir.AluOpType.bypass,
                    replica_groups=PAIRS, ins=[ar_in.opt()], outs=[ar_out.opt()],
                )
                add_residual(ar_out)
                layer_norm(lnt[:, :, 0], lnt[:, :, 1])

                # ---- FFN ----
                hT = actp.tile([128, 16, QLEN], BF, tag="hT")
                for ic in range(16):
                    w1t = wstp.tile([128, 8, 128], BF, tag="w1t")
                    nc.sync.dma_start(w1t[:], w1_in[l, ic])
                    ps = psP.tile([128, QLEN], DT, tag="proj")
                    for dc in range(8):
                        nc.tensor.matmul(
                            ps[:], w1t[:, dc, :], x_bf[:, dc, :],
                            start=(dc == 0), stop=(dc == 7),
                        )
                    nc.scalar.activation(
                        hT[:, ic, :], ps[:], mybir.ActivationFunctionType.Relu,
                        bias=b1_t[:, ic : ic + 1],
                    )
                ar_in2 = dramp.tile([D_MODEL, QLEN], BF, tag="arin")
                ar_out2 = dramp.tile([2, D_MODEL, QLEN], BF, tag="arout")
                ar4b = ar_in2.rearrange("(c p) i -> p c i", p=128)
                for dmc in range(8):
                    w2t = wstp.tile([128, 16, 128], BF, tag="w2t")
                    nc.sync.dma_start(w2t[:], w2_in[l, dmc])
                    ops = psP.tile([128, QLEN], DT, tag="proj")
                    for ic in range(16):
                        nc.tensor.matmul(
                            ops[:], w2t[:, ic, :], hT[:, ic, :],
                            start=(ic == 0), stop=(ic == 15),
                        )
                    osb = arp.tile([128, QLEN], BF, tag="osb")
                    nc.vector.tensor_copy(osb[:], ops[:])
                    nc.sync.dma_start(ar4b[:, dmc, :], osb[:])
                nc.gpsimd.collective_compute(
                    "AllGather", mybir.AluOpType.bypass,
                    replica_groups=PAIRS, ins=[ar_in2.opt()], outs=[ar_out2.opt()],
                )
                add_residual(ar_out2, b2_t=lnt[:, :, 4])
                layer_norm(lnt[:, :, 2], lnt[:, :, 3])

            # ---- final hidden out + unembed partials ----
            nc.sync.dma_start(xout[:], x_bf[:])
            for vt in range(NVT):
                et = wstp.tile([128, 8, VT], BF, tag="et")
                nc.sync.dma_start(et[:], embT_in[vt])
                for qc in range(4):
                    lps = psP.tile([128, QLEN], DT, tag="proj")
                    for dc in range(8):
                        nc.tensor.matmul(
                            lps[:, 0:VT],
                            x_bf[:, dc, 128 * qc : 128 * qc + 128],
                            et[:, dc, :],
                            start=(dc == 0), stop=(dc == 7),
                        )
                    nc.vector.tensor_reduce(
                        lmax_sb[:, qc, vt : vt + 1], lps[:, 0:VT],
                        mybir.AxisListType.X, mybir.AluOpType.max,
                    )
                    negm = smp.tile([128, 1], DT, tag="negm")
                    nc.vector.tensor_scalar_mul(
                        negm[:], lmax_sb[:, qc, vt : vt + 1], -1.0
                    )
                    lsc = smp.tile([128, VT], BF, tag="lsc")
                    nc.scalar.activation(
                        lsc[:], lps[:, 0:VT], mybir.ActivationFunctionType.Exp,
                        bias=negm[:], accum_out=lsum_sb[:, qc, vt : vt + 1],
                    )
            nc.sync.dma_start(lmax_out[:], lmax_sb[:])
            nc.sync.dma_start(lsum_out[:], lsum_sb[:])

    nc.compile()
    return nc


def _get_nc():
    if "nc" not in _CACHE:
        _CACHE["nc"] = _build()
    return _CACHE["nc"]


def _make_pos():
    pos_seq = np.arange(KLEN - 1, -1, -1, dtype=F32)
    inv_freq = 1.0 / (10000.0 ** (np.arange(0, D_MODEL, 2, dtype=F32) / D_MODEL))
    sin_inp = np.outer(pos_seq, inv_freq).astype(F32)
    return np.concatenate([np.sin(sin_inp), np.cos(sin_inp)], -1).astype(F32)


def _prep_inputs(data, memory, emb, Wq, Wkv, Wr, Wo, ffW1, ffb1, ffW2, ffb2,
                 ln1_g, ln1_b, ln2_g, ln2_b, bias_w, bias_r):
    pos = _make_pos()                                  # [KLEN, D_MODEL]
    rk = np.einsum("kd,ldn->lkn", pos, Wr.astype(F32))  # [L, KLEN, 2*NDH]
    embT = np.ascontiguousarray(emb.T).astype(BF16)    # [D_MODEL, VOCAB]
    bwf = bias_w.reshape(-1).astype(F32)
    brf = bias_r.reshape(-1).astype(F32)

    def chunk(w, c):
        # [L, D, N] -> [L, 128, c, N] with row index = 128*ci + p
        L_, D_, N_ = w.shape
        return np.ascontiguousarray(
            w.reshape(L_, c, 128, N_).transpose(0, 2, 1, 3)).astype(BF16)

    def percol(v):
        # [L, D] -> [L, 128, D//128] per-partition layout
        return np.ascontiguousarray(
            v.reshape(L, -1, 128).transpose(0, 2, 1)).astype(F32)

    in_maps = []
    for c in range(NCORES):
        b, h = c // 2, c % 2
        nds = slice(NDH * h, NDH * h + NDH)
        dis = slice(DIH * h, DIH * h + DIH)
        rkTh = np.ascontiguousarray(
            rk[:, :, nds].transpose(0, 2, 1).reshape(L, 4, 128, KLEN)
        ).astype(BF16)
        memTb = np.ascontiguousarray(memory[:, b].transpose(0, 2, 1))  # [L,1024,512]
        embTh = embT[:, VSH * h : VSH * h + VSH]                       # [1024, VSH]
        embT4 = np.ascontiguousarray(
            embTh.reshape(8, 128, NVT, VT).transpose(2, 1, 0, 3))      # [NVT,128,8,VT]
        x0 = emb[np.asarray(data[b])].astype(F32)                      # [512, 1024]
        x0T = np.ascontiguousarray(
            x0.T.reshape(8, 128, QLEN).transpose(1, 0, 2)).astype(BF16)
        w1h = ffW1[:, :, dis]                                          # [L, 1024, 2048]
        w1g = np.ascontiguousarray(
            w1h.reshape(L, 8, 128, 16, 128).transpose(0, 3, 2, 1, 4)).astype(BF16)
        w2h = ffW2[:, dis, :]                                          # [L, 2048, 1024]
        w2g = np.ascontiguousarray(
            w2h.reshape(L, 16, 128, 8, 128).transpose(0, 3, 2, 1, 4)).astype(BF16)
        in_maps.append({
            "x0": x0T,
            "memT": chunk(memTb, 8),
            "wq": chunk(Wq[:, :, nds], 8),
            "wk": chunk(Wkv[:, :, nds], 8),
            "wv": chunk(Wkv[:, :, D_MODEL + NDH * h : D_MODEL + NDH * h + NDH], 8),
            "rkT": rkTh,
            "wo": chunk(Wo[:, nds, :], 4),
            "w1": w1g,
            "w2": w2g,
            "b1": np.ascontiguousarray(
                ffb1[:, dis].reshape(L, 16, 128).transpose(0, 2, 1)).astype(F32),
            "b2": percol(np.asarray(ffb2)),
            "g1": percol(np.asarray(ln1_g)),
            "bg1": percol(np.asarray(ln1_b)),
            "g2": percol(np.asarray(ln2_g)),
            "bg2": percol(np.asarray(ln2_b)),
            "bw": np.ascontiguousarray(bwf[nds].reshape(4, 128).T),
            "br": np.ascontiguousarray(brf[nds].reshape(4, 128).T),
            "embT": embT4,
        })
    return in_maps


def _combine(results, target, emb):
    nll = np.zeros((BSZ, QLEN), dtype=np.float64)
    for b in range(BSZ):
        r0, r1 = results[2 * b], results[2 * b + 1]
        lm = np.concatenate([r0["lmax"], r1["lmax"]], axis=-1).astype(np.float64)
        ls = np.concatenate([r0["lsum"], r1["lsum"]], axis=-1).astype(np.float64)
        M = lm.max(-1)                                   # [128, 4]
        Z = (ls * np.exp(lm - M[..., None])).sum(-1)     # [128, 4]
        logZ = (M + np.log(Z)).transpose(1, 0).reshape(QLEN)  # i = 128*qc + p
        # xout[p, dc, i] = x[i, 128 dc + p]
        xf = r0["xout"].astype(np.float64).transpose(2, 1, 0).reshape(QLEN, D_MODEL)
        et = emb[np.asarray(target[b])].astype(BF16).astype(np.float64)
        tgt = (xf * et).sum(-1)
        nll[b] = logZ - tgt
    return nll.astype(F32).reshape(-1).reshape(QLEN, BSZ)


def _prep_all(inputs):
    return _prep_inputs(
        np.asarray(inputs["data"]), np.asarray(inputs["memory"], dtype=F32),
        np.asarray(inputs["emb"], dtype=F32),
        np.asarray(inputs["Wq"], dtype=F32), np.asarray(inputs["Wkv"], dtype=F32),
        np.asarray(inputs["Wr"], dtype=F32), np.asarray(inputs["Wo"], dtype=F32),
        np.asarray(inputs["ffW1"], dtype=F32), np.asarray(inputs["ffb1"], dtype=F32),
        np.asarray(inputs["ffW2"], dtype=F32), np.asarray(inputs["ffb2"], dtype=F32),
        np.asarray(inputs["ln1_g"], dtype=F32), np.asarray(inputs["ln1_b"], dtype=F32),
        np.asarray(inputs["ln2_g"], dtype=F32), np.asarray(inputs["ln2_b"], dtype=F32),
        np.asarray(inputs["bias_w"], dtype=F32), np.asarray(inputs["bias_r"], dtype=F32),
    )


def kernel(**inputs):
    nc = _get_nc()
    target = np.asarray(inputs["target"])
    emb = np.asarray(inputs["emb"], dtype=F32)
    in_maps = _prep_all(inputs)
    res = run_bass_kernel_spmd(nc, in_maps, core_ids=list(range(NCORES)))
    return _combine(res.results, target, emb)


# revision 28
# speedup vs baseline: 1.0390x; 1.0390x over previous
"""MemTransformerLM (Transformer-XL) forward pass on 8 TRN2 NeuronCores.

Sharding: core c handles batch b = c//2 and tensor-parallel half h = c%2
(heads 8h..8h+8 of 16; FFN inner columns 2048h..2048h+2048 of 4096; vocab
16000h.. for the final logsumexp). Pairwise AllGather + local add after the
attention output projection and after FFN2.

Layout: the residual stream lives TRANSPOSED in SBUF as x[p, dc, i] =
x[i, 128*dc+p] (d on partitions), bf16. All projections consume it directly
as the matmul moving operand; attention scores are computed transposed
(scoreT[j, i], klen on partitions) so softmax probs feed PV without any
transpose. Softmax is unnormalized: exp(score*scale) accumulates through PV
and the out-projection input is scaled by 1/den per head beforehand.

rel_shift: pre[i, jj] = (q+br)_i . rk_jj is written to DRAM with SHINGLED
rows (row i at element offset i*1025 + 1). Then the plain dense [512, 1024]
view at element offset 512 satisfies dense[i, j] = pre[i, j + 511 - i] =
shifted BD, and a transpose-DMA of that view yields BD^T[j, i] directly.
Positions j > i + 512 read garbage; affine_select fills them with -1e30
(the causal mask), so exp gives exactly 0.

LayerNorm in transposed layout: token sums via ones-matmul into PSUM[1, i],
broadcast back across partitions, elementwise apply with per-partition g/b.

All matmuls bf16 with fp32 PSUM accumulation.
"""

import numpy as np
import ml_dtypes

import concourse.bass as bass
import concourse.mybir as mybir
import concourse.tile as tile
from concourse import bacc
from concourse.bass_utils import run_bass_kernel_spmd

# Model dims (hardcoded per problem spec)
L = 6
D_MODEL = 1024
D_HEAD = 64
D_INNER = 4096
BSZ = 4
QLEN = 512
MLEN = 512
KLEN = MLEN + QLEN
VOCAB = 32000
SCALE = 1.0 / (D_HEAD ** 0.5)
EPS = 1e-5
NEG = -1e30

NCORES = 8
NDH = 512          # nd per core (8 heads x 64)
DIH = 2048         # ffn inner per core
VSH = VOCAB // 2   # vocab per core (split across the pair)
VT = 400           # vocab tile width
NVT = VSH // VT    # 40

# shingled scratch: row i of pre written at element offset 1 + i*1025
SCR_N = 512 * 1025 + 1024 + 64

DT = mybir.dt.float32
BF = mybir.dt.bfloat16
F32 = np.float32
BF16 = ml_dtypes.bfloat16

PAIRS = [[0, 1], [2, 3], [4, 5], [6, 7]]

DEBUG = False  # add layer-0 intermediate dumps
DBG_P = 0   # which head-pair the bd/ex/pvr taps target
DBG_HH = 0

_CACHE: dict = {}


def _build():
    nc = bacc.Bacc("TRN2", target_bir_lowering=False, debug=False, num_devices=NCORES)

    # ---- I/O ----
    x0_in = nc.dram_tensor("x0", [128, 8, QLEN], BF, kind="ExternalInput")
    memT_in = nc.dram_tensor("memT", [L, 128, 8, MLEN], BF, kind="ExternalInput")
    wq_in = nc.dram_tensor("wq", [L, 128, 8, NDH], BF, kind="ExternalInput")
    wk_in = nc.dram_tensor("wk", [L, 128, 8, NDH], BF, kind="ExternalInput")
    wv_in = nc.dram_tensor("wv", [L, 128, 8, NDH], BF, kind="ExternalInput")
    rkT_in = nc.dram_tensor("rkT", [L, 4, 128, KLEN], BF, kind="ExternalInput")
    wo_in = nc.dram_tensor("wo", [L, 128, 4, D_MODEL], BF, kind="ExternalInput")
    # w1 regrouped per inner-chunk: [L, 16, 128, 8, 128]
    w1_in = nc.dram_tensor("w1", [L, 16, 128, 8, 128], BF, kind="ExternalInput")
    # w2 regrouped per dmodel-chunk: [L, 8, 128, 16, 128]
    w2_in = nc.dram_tensor("w2", [L, 8, 128, 16, 128], BF, kind="ExternalInput")
    b1_in = nc.dram_tensor("b1", [L, 128, 16], DT, kind="ExternalInput")
    # packed per-partition params: [g1, bg1, g2, bg2, b2] along dim 2
    lns_in = nc.dram_tensor("lns", [L, 128, 5, 8], DT, kind="ExternalInput")
    bw_in = nc.dram_tensor("bw", [128, 4], DT, kind="ExternalInput")
    br_in = nc.dram_tensor("br", [128, 4], DT, kind="ExternalInput")
    embT_in = nc.dram_tensor("embT", [NVT, 128, 8, VT], BF, kind="ExternalInput")

    xout = nc.dram_tensor("xout", [128, 8, QLEN], BF, kind="ExternalOutput")
    lmax_out = nc.dram_tensor("lmax", [128, 4, NVT], DT, kind="ExternalOutput")
    lsum_out = nc.dram_tensor("lsum", [128, 4, NVT], DT, kind="ExternalOutput")
    if DEBUG:
        dbg_pv = nc.dram_tensor("dbg_pv", [128, 4, QLEN], DT, kind="ExternalOutput")
        dbg_bd = nc.dram_tensor("dbg_bd", [128, 8, QLEN], DT, kind="ExternalOutput")
        dbg_ex = nc.dram_tensor("dbg_ex", [128, 8, QLEN], DT, kind="ExternalOutput")
        dbg_x1 = nc.dram_tensor("dbg_x1", [128, 8, QLEN], DT, kind="ExternalOutput")
        dbg_kv = nc.dram_tensor("dbg_kv", [128, 8, QLEN], DT, kind="ExternalOutput")
        dbg_rb = nc.dram_tensor("dbg_rb", [128, QLEN], DT, kind="ExternalOutput")
        dbg_pvr = nc.dram_tensor("dbg_pvr", [128, QLEN], DT, kind="ExternalOutput")

    from contextlib import ExitStack
    with tile.TileContext(nc) as tc:
        with ExitStack() as stack:
            ep = stack.enter_context
            constp = ep(tc.tile_pool(name="const", bufs=1))
            resp = ep(tc.tile_pool(name="res", bufs=1))
            wtp = ep(tc.tile_pool(name="wts", bufs=1))
            wstp = ep(tc.tile_pool(name="wst", bufs=3))   # streamed weight tiles
            actp = ep(tc.tile_pool(name="act", bufs=1))
            prqp = ep(tc.tile_pool(name="prq", bufs=2))   # per-pair q/k/rk tiles
            hdp = ep(tc.tile_pool(name="hd", bufs=2))     # per-head bd tiles
            hd1p = ep(tc.tile_pool(name="hd1", bufs=1))   # exp/esum/recb tiles
            prep = ep(tc.tile_pool(name="pre2", bufs=2))  # pre SBUF staging
            lnp = ep(tc.tile_pool(name="lnp", bufs=1))    # LN scratch
            smp = ep(tc.tile_pool(name="small", bufs=2))
            arp = ep(tc.tile_pool(name="arp", bufs=2))    # collective readback
            psA = ep(tc.tile_pool(name="ps_ac", bufs=3, space="PSUM"))
            psSp = ep(tc.tile_pool(name="ps_pre", bufs=2, space="PSUM"))
            psV = ep(tc.tile_pool(name="ps_pv", bufs=1, space="PSUM"))
            psP = ep(tc.tile_pool(name="ps_proj", bufs=2, space="PSUM"))
            dramp = ep(tc.tile_pool(name="dram", bufs=2, space="DRAM"))
            bw_t = constp.tile([128, 4], DT)
            br_t = constp.tile([128, 4], DT)
            ones_t = constp.tile([128, 1], BF)
            nc.sync.dma_start(bw_t[:], bw_in[:])
            nc.sync.dma_start(br_t[:], br_in[:])
            nc.vector.memset(ones_t[:], 1.0)

            # residual stream, bf16, transposed: x[p, dc, i] = x[i, 128 dc + p]
            x_bf = resp.tile([128, 8, QLEN], BF)
            nc.sync.dma_start(x_bf[:], x0_in[:])
            lmax_sb = resp.tile([128, 4, NVT], DT)
            lsum_sb = resp.tile([128, 4, NVT], DT)

            def layer_norm(g_t, b_t):
                """LN over d (partitions x 8 chunks) of x_bf, in place."""
                sq = actp.tile([128, 8, QLEN], BF, tag="hT")  # alias hT slot
                for dc in range(8):
                    nc.vector.tensor_tensor(
                        sq[:, dc, :], x_bf[:, dc, :], x_bf[:, dc, :],
                        mybir.AluOpType.mult,
                    )
                xs_ps = psP.tile([1, QLEN], DT, tag="proj")
                for dc in range(8):
                    nc.tensor.matmul(
                        xs_ps[:], ones_t[:], x_bf[:, dc, :],
                        start=(dc == 0), stop=(dc == 7),
                    )
                sq_ps = psP.tile([1, QLEN], DT, tag="proj")
                for dc in range(8):
                    nc.tensor.matmul(
                        sq_ps[:], ones_t[:], sq[:, dc, :],
                        start=(dc == 0), stop=(dc == 7),
                    )
                xs_sb = smp.tile([1, QLEN], DT, tag="xs")
                sq_sb = smp.tile([1, QLEN], DT, tag="sqs")
                nc.vector.tensor_copy(xs_sb[:], xs_ps[:])
                nc.vector.tensor_copy(sq_sb[:], sq_ps[:])
                XS = lnp.tile([128, QLEN], DT, tag="XS")
                SQ = lnp.tile([128, QLEN], DT, tag="SQ")
                nc.gpsimd.partition_broadcast(XS[:], xs_sb[:])
                nc.gpsimd.partition_broadcast(SQ[:], sq_sb[:])
                mu = lnp.tile([128, QLEN], DT, tag="mu")
                var = lnp.tile([128, QLEN], DT, tag="var")
                rstd = lnp.tile([128, QLEN], DT, tag="rstd")
                nc.vector.tensor_scalar_mul(mu[:], XS[:], 1.0 / D_MODEL)
                nc.vector.tensor_tensor(var[:], mu[:], mu[:], mybir.AluOpType.mult)
                nc.vector.tensor_scalar(
                    SQ[:], SQ[:], 1.0 / D_MODEL, EPS,
                    mybir.AluOpType.mult, mybir.AluOpType.add,
                )
                nc.vector.tensor_tensor(
                    var[:], SQ[:], var[:], mybir.AluOpType.subtract
                )
                nc.scalar.sqrt(var[:], var[:])
                nc.vector.reciprocal(rstd[:], var[:])
                for dc in range(8):
                    xc = lnp.tile([128, QLEN], DT, tag="xc")
                    nc.vector.tensor_tensor(
                        xc[:], x_bf[:, dc, :], mu[:], mybir.AluOpType.subtract
                    )
                    nc.vector.tensor_tensor(
                        xc[:], xc[:], rstd[:], mybir.AluOpType.mult
                    )
                    nc.vector.tensor_scalar(
                        x_bf[:, dc, :], xc[:], g_t[:, dc : dc + 1],
                        b_t[:, dc : dc + 1],
                        mybir.AluOpType.mult, mybir.AluOpType.add,
                    )

            def add_residual(ar_out, b2_t=None):
                """x_bf += allgathered partial sums (+ b2)."""
                arr4 = ar_out.rearrange("r (c p) i -> r p c i", p=128)
                for dc in range(8):
                    arr = arp.tile([128, QLEN], BF, tag="arr")
                    nc.sync.dma_start(arr[:], arr4[0, :, dc, :])
                    nc.gpsimd.dma_start(
                        arr[:], arr4[1, :, dc, :], accum_op=mybir.AluOpType.add
                    )
                    if b2_t is not None:
                        nc.vector.tensor_scalar(
                            arr[:], arr[:], b2_t[:, dc : dc + 1], None,
                            mybir.AluOpType.add,
                        )
                    nc.vector.tensor_tensor(
                        x_bf[:, dc, :], x_bf[:, dc, :], arr[:], mybir.AluOpType.add
                    )

            for l in range(L):
                # ---- whole-layer loads ----
                wq_t = wtp.tile([128, 8, NDH], BF, tag="wq")
                wk_t = wtp.tile([128, 8, NDH], BF, tag="wk")
                wv_t = wtp.tile([128, 8, NDH], BF, tag="wv")
                nc.sync.dma_start(wq_t[:], wq_in[l])
                nc.sync.dma_start(wk_t[:], wk_in[l])
                nc.sync.dma_start(wv_t[:], wv_in[l])
                memT_t = actp.tile([128, 8, MLEN], BF, tag="memT")
                nc.sync.dma_start(memT_t[:], memT_in[l])
                b1_t = wtp.tile([128, 16], DT, tag="b1")
                nc.sync.dma_start(b1_t[:], b1_in[l])
                lnt = wtp.tile([128, 5, 8], DT, tag="lns")
                nc.sync.dma_start(lnt[:], lns_in[l])

                # ---- v projection: vv[p, kc, n] = v[128 kc + p, n], all heads ----
                vv = actp.tile([128, 8, NDH], BF, tag="vv")
                for kc in range(8):
                    vps = psP.tile([128, NDH], DT, tag="proj")
                    src = memT_t if kc < 4 else x_bf
                    ksl = slice(128 * (kc % 4), 128 * (kc % 4) + 128)
                    for dc in range(8):
                        nc.tensor.matmul(
                            vps[:], src[:, dc, ksl], wv_t[:, dc, :],
                            start=(dc == 0), stop=(dc == 7),
                        )
                    nc.vector.tensor_copy(vv[:, kc, :], vps[:])

                pvT_all = actp.tile([128, 4, QLEN], BF, tag="pvT")

                # ---- attention, one 2-head pair (= one 128-row nd chunk) at a time
                for p in range(4):
                    nsl = slice(128 * p, 128 * p + 128)
                    # q^T + biases
                    qps = psP.tile([128, QLEN], DT, tag="proj")
                    for dc in range(8):
                        nc.tensor.matmul(
                            qps[:], wq_t[:, dc, nsl], x_bf[:, dc, :],
                            start=(dc == 0), stop=(dc == 7),
                        )
                    qbwT = prqp.tile([128, QLEN], BF, tag="qbw")
                    qbrT = prqp.tile([128, QLEN], BF, tag="qbr")
                    nc.scalar.add(qbwT[:], qps[:], bw_t[:, p : p + 1])
                    nc.scalar.add(qbrT[:], qps[:], br_t[:, p : p + 1])
                    # k^T chunk: kT[p2, j] with p2 = nd within chunk
                    kT = prqp.tile([128, KLEN], BF, tag="kT")
                    for kh in range(2):
                        kps = psP.tile([128, QLEN], DT, tag="proj")
                        src = memT_t if kh == 0 else x_bf
                        for dc in range(8):
                            nc.tensor.matmul(
                                kps[:], wk_t[:, dc, nsl], src[:, dc, :],
                                start=(dc == 0), stop=(dc == 7),
                            )
                        nc.vector.tensor_copy(kT[:, 512 * kh : 512 * kh + 512], kps[:])
                    rkT = prqp.tile([128, KLEN], BF, tag="rkT")
                    nc.sync.dma_start(rkT[:], rkT_in[l, p])

                    # pre = (q+br)^T-chunk @ rkT per head, shingled-write to DRAM
                    scrs = []
                    for hh in range(2):
                        scr = dramp.tile([SCR_N], BF, tag=f"scr{hh}")
                        scrs.append(scr)
                    for ic in range(4):
                        pre_sb = prep.tile([128, 2, KLEN], BF, tag="pre_sb")
                        for hh in range(2):
                            base = 64 * hh
                            for jh in range(2):
                                pps = psSp.tile([128, 512], DT, tag="pre")
                                nc.tensor.matmul(
                                    pps[:],
                                    qbrT[base : base + 64, 128 * ic : 128 * ic + 128],
                                    rkT[base : base + 64, 512 * jh : 512 * jh + 512],
                                    start=True, stop=True,
                                )
                                nc.vector.tensor_copy(
                                    pre_sb[:, hh, 512 * jh : 512 * jh + 512], pps[:]
                                )
                        for hh in range(2):
                            shingle = bass.AP(
                                scrs[hh].tensor,
                                scrs[hh].offset + 1 + 128 * ic * 1025,
                                [[1025, 128], [1, KLEN]],
                            )
                            nc.sync.dma_start(shingle, pre_sb[:, hh, :])

                    # per head: transpose-read shifted BD^T, mask, AC, exp, PV
                    pv = psV.tile([128, QLEN], DT, tag="pv")
                    for hh in range(2):
                        base = 64 * hh
                        h2 = 2 * p + hh
                        scr = scrs[hh]
                        dense = scr[512 : 512 + QLEN * KLEN].rearrange(
                            "(i j) -> i j", j=KLEN
                        )
                        bd = hdp.tile([128, 8, QLEN], BF, tag=f"bd{hh}")
                        nc.scalar.dma_start_transpose(bd[:, 0:4, :], dense[:, 0:512])
                        nc.scalar.dma_start_transpose(bd[:, 4:8, :], dense[:, 512:])
                        # mask: keep i >= 128 jc + pj - 512 (only jc>=4 can mask)
                        for jc in range(4, 8):
                            w = 128 * (jc - 3)
                            nc.gpsimd.affine_select(
                                out=bd[:, jc, 0:w], in_=bd[:, jc, 0:w],
                                pattern=[[1, w]],
                                compare_op=mybir.AluOpType.is_ge,
                                fill=NEG, base=512 - 128 * jc,
                                channel_multiplier=-1,
                            )
                        if DEBUG and l == 0 and p == DBG_P and hh == DBG_HH:
                            nc.gpsimd.dma_start(dbg_bd[:], bd[:])
                        expT = hd1p.tile([128, 8, QLEN], BF, tag=f"expT{hh}")
                        for jc in range(8):
                            acp = psA.tile([128, QLEN], DT, tag="ac")
                            nc.tensor.matmul(
                                acp[:],
                                kT[base : base + 64, 128 * jc : 128 * jc + 128],
                                qbwT[base : base + 64, :],
                                start=True, stop=True,
                            )
                            nc.vector.tensor_tensor(
                                acp[:], acp[:], bd[:, jc, :], mybir.AluOpType.add
                            )
                            nc.scalar.activation(
                                expT[:, jc, :], acp[:],
                                mybir.ActivationFunctionType.Exp, scale=SCALE,
                            )
                        if DEBUG and l == 0 and p == DBG_P and hh == DBG_HH:
                            nc.gpsimd.dma_start(dbg_ex[:], expT[:])
                        # denominator: sum over j = chunk-add then ones-matmul
                        esum = hd1p.tile([128, QLEN], BF, tag="esum")
                        nc.vector.tensor_tensor(
                            esum[:], expT[:, 0, :], expT[:, 1, :], mybir.AluOpType.add
                        )
                        for jc in range(2, 8):
                            nc.vector.tensor_tensor(
                                esum[:], esum[:], expT[:, jc, :], mybir.AluOpType.add
                            )
                        den_ps = psSp.tile([1, QLEN], DT, tag="pre")
                        nc.tensor.matmul(
                            den_ps[:], ones_t[:], esum[:], start=True, stop=True
                        )
                        rec = smp.tile([1, QLEN], DT, tag="rec")
                        nc.vector.reciprocal(rec[:], den_ps[:])
                        # broadcast from partition 0 requires out base 0: fill
                        # all 128 partitions, use the lane-aligned half below
                        recb = hd1p.tile([128, QLEN], DT, tag=f"recb{hh}")
                        nc.gpsimd.partition_broadcast(recb[:], rec[:])
                        # PV: vecT_unnorm[d, i]; head A -> psum rows 0:64,
                        # head B -> rows 64:128 (col-group tiling)
                        for jc in range(8):
                            nc.tensor.matmul(
                                pv[base : base + 64, :],
                                vv[:, jc, 64 * h2 : 64 * h2 + 64],
                                expT[:, jc, :],
                                start=(jc == 0), stop=(jc == 7),
                                tile_position=(0, base),
                            )
                        nc.vector.tensor_tensor(
                            pvT_all[base : base + 64, p, :],
                            pv[base : base + 64, :], recb[base : base + 64, :],
                            mybir.AluOpType.mult,
                        )
                        if DEBUG and l == 0 and p == DBG_P and hh == 1:
                            pvsb = hd1p.tile([128, QLEN], DT, tag="dbgpv")
                            nc.vector.tensor_copy(pvsb[:], pv[:])
                            nc.gpsimd.dma_start(dbg_pvr[:], pvsb[:])
                            nc.gpsimd.dma_start(dbg_rb[:], recb[:])

                if DEBUG and l == 0:
                    nc.gpsimd.dma_start(dbg_pv[:], pvT_all[:])
                    nc.gpsimd.dma_start(dbg_kv[:], vv[:])
                # ---- attention out projection (transposed) + pairwise exchange
                wo_t = wtp.tile([128, 4, D_MODEL], BF, tag="wq")  # alias wq slot
                nc.sync.dma_start(wo_t[:], wo_in[l])
                ar_in = dramp.tile([D_MODEL, QLEN], BF, tag="arin")
                ar_out = dramp.tile([2, D_MODEL, QLEN], BF, tag="arout")
                ar4 = ar_in.rearrange("(c p) i -> p c i", p=128)
                for dmc in range(8):
                    ops = psP.tile([128, QLEN], DT, tag="proj")
                    for p in range(4):
                        nc.tensor.matmul(
                            ops[:],
                            wo_t[:, p, 128 * dmc : 128 * dmc + 128],
                            pvT_all[:, p, :],
                            start=(p == 0), stop=(p == 3),
                        )
                    osb = arp.tile([128, QLEN], BF, tag="osb")
                    nc.vector.tensor_copy(osb[:], ops[:])
                    nc.sync.dma_start(ar4[:, dmc, :], osb[:])
                nc.gpsimd.collective_compute(
                    "AllGather", mybir.AluOpType.bypass,
                    replica_groups=PAIRS, ins=[ar_in.opt()], outs=[ar_out.opt()],
                )
                add_residual(ar_out)
                layer_norm(lnt[:, 0, :], lnt[:, 1, :])
                if DEBUG and l == 0:
                    nc.gpsimd.dma_start(dbg_x1[:], x_bf[:])

                # ---- FFN ----
                hT = actp.tile([128, 16, QLEN], BF, tag="hT")
                for ic in range(16):
                    w1t = wstp.tile([128, 8, 128], BF, tag="w1t")
                    nc.sync.dma_start(w1t[:], w1_in[l, ic])
                    ps = psP.tile([128, QLEN], DT, tag="proj")
                    for dc in range(8):
                        nc.tensor.matmul(
                            ps[:], w1t[:, dc, :], x_bf[:, dc, :],
                            start=(dc == 0), stop=(dc == 7),
                        )
                    nc.scalar.activation(
                        hT[:, ic, :], ps[:], mybir.ActivationFunctionType.Relu,
                        bias=b1_t[:, ic : ic + 1],
                    )
                ar_in2 = dramp.tile([D_MODEL, QLEN], BF, tag="arin")
                ar_out2 = dramp.tile([2, D_MODEL, QLEN], BF, tag="arout")
                ar4b = ar_in2.rearrange("(c p) i -> p c i", p=128)
                for dmc in range(8):
                    w2t = wstp.tile([128, 16, 128], BF, tag="w2t")
                    nc.sync.dma_start(w2t[:], w2_in[l, dmc])
                    ops = psP.tile([128, QLEN], DT, tag="proj")
                    for ic in range(16):
                        nc.tensor.matmul(
                            ops[:], w2t[:, ic, :], hT[:, ic, :],
                            start=(ic == 0), stop=(ic == 15),
                        )
                    osb = arp.tile([128, QLEN], BF, tag="osb")
                    nc.vector.tensor_copy(osb[:], ops[:])
                    nc.sync.dma_start(ar4b[:, dmc, :], osb[:])
                nc.gpsimd.collective_compute(
                    "AllGather", mybir.AluOpType.bypass,
                    replica_groups=PAIRS, ins=[ar_in2.opt()], outs=[ar_out2.opt()],
                )
                add_residual(ar_out2, b2_t=lnt[:, 4, :])
                layer_norm(lnt[:, 2, :], lnt[:, 3, :])

            # ---- final hidden out + unembed partials ----
            nc.sync.dma_start(xout[:], x_bf[:])
            for vt in range(NVT):
                # double-buffer embT tiles in the dead wk/wv weight slots
                et = wtp.tile([128, 8, VT], BF, tag=("wk" if vt % 2 == 0 else "wv"))
                nc.sync.dma_start(et[:], embT_in[vt])
                for qc in range(4):
                    lps = psP.tile([128, QLEN], DT, tag="proj")
                    for dc in range(8):
                        nc.tensor.matmul(
                            lps[:, 0:VT],
                            x_bf[:, dc, 128 * qc : 128 * qc + 128],
                            et[:, dc, :],
                            start=(dc == 0), stop=(dc == 7),
                        )
                    nc.vector.tensor_reduce(
                        lmax_sb[:, qc, vt : vt + 1], lps[:, 0:VT],
                        mybir.AxisListType.X, mybir.AluOpType.max,
                    )
                    negm = smp.tile([128, 1], DT, tag="negm")
                    nc.vector.tensor_scalar_mul(
                        negm[:], lmax_sb[:, qc, vt : vt + 1], -1.0
                    )
                    lsc = smp.tile([128, VT], BF, tag="lsc")
                    nc.scalar.activation(
                        lsc[:], lps[:, 0:VT], mybir.ActivationFunctionType.Exp,
                        bias=negm[:], accum_out=lsum_sb[:, qc, vt : vt + 1],
                    )
            nc.sync.dma_start(lmax_out[:], lmax_sb[:])
            nc.sync.dma_start(lsum_out[:], lsum_sb[:])

    nc.compile()
    return nc


def _get_nc():
    if "nc" not in _CACHE:
        _CACHE["nc"] = _build()
    return _CACHE["nc"]


def _make_pos():
    pos_seq = np.arange(KLEN - 1, -1, -1, dtype=F32)
    inv_freq = 1.0 / (10000.0 ** (np.arange(0, D_MODEL, 2, dtype=F32) / D_MODEL))
    sin_inp = np.outer(pos_seq, inv_freq).astype(F32)
    return np.concatenate([np.sin(sin_inp), np.cos(sin_inp)], -1).astype(F32)


def _prep_inputs(data, memory, emb, Wq, Wkv, Wr, Wo, ffW1, ffb1, ffW2, ffb2,
                 ln1_g, ln1_b, ln2_g, ln2_b, bias_w, bias_r):
    # honor a reduced layer count (debugging)
    memory, Wq, Wkv, Wr, Wo = memory[:L], Wq[:L], Wkv[:L], Wr[:L], Wo[:L]
    ffW1, ffb1, ffW2, ffb2 = ffW1[:L], ffb1[:L], ffW2[:L], ffb2[:L]
    ln1_g, ln1_b, ln2_g, ln2_b = ln1_g[:L], ln1_b[:L], ln2_g[:L], ln2_b[:L]
    pos = _make_pos()                                  # [KLEN, D_MODEL]
    rk = np.einsum("kd,ldn->lkn", pos, Wr.astype(F32))  # [L, KLEN, 2*NDH]
    embT = np.ascontiguousarray(emb.T).astype(BF16)    # [D_MODEL, VOCAB]
    bwf = bias_w.reshape(-1).astype(F32)
    brf = bias_r.reshape(-1).astype(F32)

    def chunk(w, c):
        # [L, D, N] -> [L, 128, c, N] with row index = 128*ci + p
        L_, D_, N_ = w.shape
        return np.ascontiguousarray(
            w.reshape(L_, c, 128, N_).transpose(0, 2, 1, 3)).astype(BF16)

    def percol(v):
        # [L, D] -> [L, 128, D//128] per-partition layout
        return np.ascontiguousarray(
            v.reshape(L, -1, 128).transpose(0, 2, 1)).astype(F32)

    in_maps = []
    for c in range(NCORES):
        b, h = c // 2, c % 2
        nds = slice(NDH * h, NDH * h + NDH)
        dis = slice(DIH * h, DIH * h + DIH)
        rkTh = np.ascontiguousarray(
            rk[:, :, nds].transpose(0, 2, 1).reshape(L, 4, 128, KLEN)
        ).astype(BF16)
        memTb = np.ascontiguousarray(memory[:, b].transpose(0, 2, 1))  # [L,1024,512]
        embTh = embT[:, VSH * h : VSH * h + VSH]                       # [1024, VSH]
        embT4 = np.ascontiguousarray(
            embTh.reshape(8, 128, NVT, VT).transpose(2, 1, 0, 3))      # [NVT,128,8,VT]
        x0 = emb[np.asarray(data[b])].astype(F32)                      # [512, 1024]
        x0T = np.ascontiguousarray(
            x0.T.reshape(8, 128, QLEN).transpose(1, 0, 2)).astype(BF16)
        w1h = ffW1[:, :, dis]                                          # [L, 1024, 2048]
        w1g = np.ascontiguousarray(
            w1h.reshape(L, 8, 128, 16, 128).transpose(0, 3, 2, 1, 4)).astype(BF16)
        w2h = ffW2[:, dis, :]                                          # [L, 2048, 1024]
        w2g = np.ascontiguousarray(
            w2h.reshape(L, 16, 128, 8, 128).transpose(0, 3, 2, 1, 4)).astype(BF16)
        in_maps.append({
            "x0": x0T,
            "memT": chunk(memTb, 8),
            "wq": chunk(Wq[:, :, nds], 8),
            "wk": chunk(Wkv[:, :, nds], 8),
            "wv": chunk(Wkv[:, :, D_MODEL + NDH * h : D_MODEL + NDH * h + NDH], 8),
            "rkT": rkTh,
            "wo": chunk(Wo[:, nds, :], 4),
            "w1": w1g,
            "w2": w2g,
            "b1": np.ascontiguousarray(
                ffb1[:, dis].reshape(L, 16, 128).transpose(0, 2, 1)).astype(F32),
            "lns": np.ascontiguousarray(np.stack(
                [percol(np.asarray(v)) for v in
                 (ln1_g, ln1_b, ln2_g, ln2_b, ffb2)], axis=2)),
            "bw": np.ascontiguousarray(bwf[nds].reshape(4, 128).T),
            "br": np.ascontiguousarray(brf[nds].reshape(4, 128).T),
            "embT": embT4,
        })
    return in_maps


def _combine(results, target, emb):
    nll = np.zeros((BSZ, QLEN), dtype=np.float64)
    for b in range(BSZ):
        r0, r1 = results[2 * b], results[2 * b + 1]
        lm = np.concatenate([r0["lmax"], r1["lmax"]], axis=-1).astype(np.float64)
        ls = np.concatenate([r0["lsum"], r1["lsum"]], axis=-1).astype(np.float64)
        M = lm.max(-1)                                   # [128, 4]
        Z = (ls * np.exp(lm - M[..., None])).sum(-1)     # [128, 4]
        logZ = (M + np.log(Z)).transpose(1, 0).reshape(QLEN)  # i = 128*qc + p
        # xout[p, dc, i] = x[i, 128 dc + p]
        xf = r0["xout"].astype(np.float64).transpose(2, 1, 0).reshape(QLEN, D_MODEL)
        et = emb[np.asarray(target[b])].astype(BF16).astype(np.float64)
        tgt = (xf * et).sum(-1)
        nll[b] = logZ - tgt
    return nll.astype(F32).reshape(-1).reshape(QLEN, BSZ)


def _prep_all(inputs):
    return _prep_inputs(
        np.asarray(inputs["data"]), np.asarray(inputs["memory"], dtype=F32),
        np.asarray(inputs["emb"], dtype=F32),
        np.asarray(inputs["Wq"], dtype=F32), np.asarray(inputs["Wkv"], dtype=F32),
        np.asarray(inputs["Wr"], dtype=F32), np.asarray(inputs["Wo"], dtype=F32),
        np.asarray(inputs["ffW1"], dtype=F32), np.asarray(inputs["ffb1"], dtype=F32),
        np.asarray(inputs["ffW2"], dtype=F32), np.asarray(inputs["ffb2"], dtype=F32),
        np.asarray(inputs["ln1_g"], dtype=F32), np.asarray(inputs["ln1_b"], dtype=F32),
        np.asarray(inputs["ln2_g"], dtype=F32), np.asarray(inputs["ln2_b"], dtype=F32),
        np.asarray(inputs["bias_w"], dtype=F32), np.asarray(inputs["bias_r"], dtype=F32),
    )


def kernel(**inputs):
    nc = _get_nc()
    target = np.asarray(inputs["target"])
    emb = np.asarray(inputs["emb"], dtype=F32)
    in_maps = _prep_all(inputs)
    res = run_bass_kernel_spmd(nc, in_maps, core_ids=list(range(NCORES)))
    return _combine(res.results, target, emb)


# revision 39
# speedup vs baseline: 1.0449x; 1.0057x over previous
"""MemTransformerLM (Transformer-XL) forward pass on 8 TRN2 NeuronCores.

Sharding: core c handles batch b = c//2 and tensor-parallel half h = c%2
(heads 8h..8h+8 of 16; FFN inner columns 2048h..2048h+2048 of 4096; vocab
16000h.. for the final logsumexp). Pairwise AllGather + local add after the
attention output projection and after FFN2.

Layout: the residual stream lives TRANSPOSED in SBUF as x[p, dc, i] =
x[i, 128*dc+p] (d on partitions), bf16. All projections consume it directly
as the matmul moving operand; attention scores are computed transposed
(scoreT[j, i], klen on partitions) so softmax probs feed PV without any
transpose. Softmax is unnormalized: exp(score*scale) accumulates through PV
and the out-projection input is scaled by 1/den per head beforehand.

rel_shift: pre[i, jj] = (q+br)_i . rk_jj is written to DRAM with SHINGLED
rows (row i at element offset i*1025 + 1). Then the plain dense [512, 1024]
view at element offset 512 satisfies dense[i, j] = pre[i, j + 511 - i] =
shifted BD, and a transpose-DMA of that view yields BD^T[j, i] directly.
Positions j > i + 512 read garbage; affine_select fills them with -1e30
(the causal mask), so exp gives exactly 0.

LayerNorm in transposed layout: token sums via ones-matmul into PSUM[1, i],
broadcast back across partitions, elementwise apply with per-partition g/b.

All matmuls bf16 with fp32 PSUM accumulation.
"""

import numpy as np
import ml_dtypes

import concourse.bass as bass
import concourse.mybir as mybir
import concourse.tile as tile
from concourse import bacc
from concourse.bass_utils import run_bass_kernel_spmd

# Model dims (hardcoded per problem spec)
L = 6
D_MODEL = 1024
D_HEAD = 64
D_INNER = 4096
BSZ = 4
QLEN = 512
MLEN = 512
KLEN = MLEN + QLEN
VOCAB = 32000
SCALE = 1.0 / (D_HEAD ** 0.5)
EPS = 1e-5
NEG = -1e30

NCORES = 8
NDH = 512          # nd per core (8 heads x 64)
DIH = 2048         # ffn inner per core
VSH = VOCAB // 2   # vocab per core (split across the pair)
VT = 400           # vocab tile width
NVT = VSH // VT    # 40

# shingled scratch: row i of pre written at element offset 1 + i*1025
SCR_N = 512 * 1025 + 1024 + 64

DT = mybir.dt.float32
BF = mybir.dt.bfloat16
F32 = np.float32
BF16 = ml_dtypes.bfloat16

PAIRS = [[0, 1], [2, 3], [4, 5], [6, 7]]

DEBUG = False  # add layer-0 intermediate dumps
DBG_P = 0   # which head-pair the bd/ex/pvr taps target
DBG_HH = 0

_CACHE: dict = {}


def _build():
    nc = bacc.Bacc("TRN2", target_bir_lowering=False, debug=False, num_devices=NCORES)

    # ---- I/O ----
    x0_in = nc.dram_tensor("x0", [128, 8, QLEN], BF, kind="ExternalInput")
    memT_in = nc.dram_tensor("memT", [L, 128, 8, MLEN], BF, kind="ExternalInput")
    wq_in = nc.dram_tensor("wq", [L, 128, 8, NDH], BF, kind="ExternalInput")
    wk_in = nc.dram_tensor("wk", [L, 128, 8, NDH], BF, kind="ExternalInput")
    wv_in = nc.dram_tensor("wv", [L, 128, 8, NDH], BF, kind="ExternalInput")
    rkT_in = nc.dram_tensor("rkT", [L, 4, 128, KLEN], BF, kind="ExternalInput")
    wo_in = nc.dram_tensor("wo", [L, 128, 4, D_MODEL], BF, kind="ExternalInput")
    # w1 regrouped per inner-chunk: [L, 16, 128, 8, 128]
    w1_in = nc.dram_tensor("w1", [L, 16, 128, 8, 128], BF, kind="ExternalInput")
    # w2 regrouped per dmodel-chunk: [L, 8, 128, 16, 128]
    w2_in = nc.dram_tensor("w2", [L, 8, 128, 16, 128], BF, kind="ExternalInput")
    b1_in = nc.dram_tensor("b1", [L, 128, 16], DT, kind="ExternalInput")
    # packed per-partition params: [g1, bg1, g2, bg2, b2] along dim 2
    lns_in = nc.dram_tensor("lns", [L, 128, 5, 8], DT, kind="ExternalInput")
    bw_in = nc.dram_tensor("bw", [128, 4], DT, kind="ExternalInput")
    br_in = nc.dram_tensor("br", [128, 4], DT, kind="ExternalInput")
    embT_in = nc.dram_tensor("embT", [NVT, 128, 8, VT], BF, kind="ExternalInput")

    xout = nc.dram_tensor("xout", [128, 8, QLEN], BF, kind="ExternalOutput")
    lmax_out = nc.dram_tensor("lmax", [128, 4, NVT], DT, kind="ExternalOutput")
    lsum_out = nc.dram_tensor("lsum", [128, 4, NVT], DT, kind="ExternalOutput")
    if DEBUG:
        dbg_pv = nc.dram_tensor("dbg_pv", [128, 4, QLEN], DT, kind="ExternalOutput")
        dbg_bd = nc.dram_tensor("dbg_bd", [128, 8, QLEN], DT, kind="ExternalOutput")
        dbg_ex = nc.dram_tensor("dbg_ex", [128, 8, QLEN], DT, kind="ExternalOutput")
        dbg_x1 = nc.dram_tensor("dbg_x1", [128, 8, QLEN], DT, kind="ExternalOutput")
        dbg_kv = nc.dram_tensor("dbg_kv", [128, 8, QLEN], DT, kind="ExternalOutput")
        dbg_rb = nc.dram_tensor("dbg_rb", [128, QLEN], DT, kind="ExternalOutput")
        dbg_pvr = nc.dram_tensor("dbg_pvr", [128, QLEN], DT, kind="ExternalOutput")

    from contextlib import ExitStack
    with tile.TileContext(nc) as tc:
        with ExitStack() as stack:
            ep = stack.enter_context
            constp = ep(tc.tile_pool(name="const", bufs=1))
            resp = ep(tc.tile_pool(name="res", bufs=1))
            wtp = ep(tc.tile_pool(name="wts", bufs=1))
            wstp = ep(tc.tile_pool(name="wst", bufs=3))   # streamed weight tiles
            actp = ep(tc.tile_pool(name="act", bufs=1))
            prqp = ep(tc.tile_pool(name="prq", bufs=2))   # per-pair q/k/rk tiles
            hdp = ep(tc.tile_pool(name="hd", bufs=2))     # per-head bd tiles
            hd1p = ep(tc.tile_pool(name="hd1", bufs=1))   # exp/esum/recb tiles
            prep = ep(tc.tile_pool(name="pre2", bufs=2))  # pre SBUF staging
            lnp = ep(tc.tile_pool(name="lnp", bufs=1))    # LN scratch
            smp = ep(tc.tile_pool(name="small", bufs=2))
            arp = ep(tc.tile_pool(name="arp", bufs=2))    # collective readback
            psA = ep(tc.tile_pool(name="ps_ac", bufs=3, space="PSUM"))
            psSp = ep(tc.tile_pool(name="ps_pre", bufs=1, space="PSUM"))
            psV = ep(tc.tile_pool(name="ps_pv", bufs=1, space="PSUM"))
            psP = ep(tc.tile_pool(name="ps_proj", bufs=2, space="PSUM"))
            dramp = ep(tc.tile_pool(name="dram", bufs=2, space="DRAM"))
            bw_t = constp.tile([128, 4], DT)
            br_t = constp.tile([128, 4], DT)
            ones_t = constp.tile([128, 1], BF)
            nc.sync.dma_start(bw_t[:], bw_in[:])
            nc.sync.dma_start(br_t[:], br_in[:])
            nc.vector.memset(ones_t[:], 1.0)
            id_t = constp.tile([128, 128], BF)
            nc.vector.memset(id_t[:], 1.0)
            nc.gpsimd.affine_select(
                out=id_t[:], in_=id_t[:], pattern=[[-1, 128]],
                compare_op=mybir.AluOpType.is_equal, fill=0.0,
                base=0, channel_multiplier=1,
            )

            # residual stream, bf16, transposed: x[p, dc, i] = x[i, 128 dc + p]
            x_bf = resp.tile([128, 8, QLEN], BF)
            nc.sync.dma_start(x_bf[:], x0_in[:])
            lmax_sb = resp.tile([128, 4, NVT], DT)
            lsum_sb = resp.tile([128, 4, NVT], DT)

            def layer_norm(g_t, b_t):
                """LN over d (partitions x 8 chunks) of x_bf, in place."""
                sq = actp.tile([128, 8, QLEN], BF, tag="hT")  # alias hT slot
                for dc in range(8):
                    nc.vector.tensor_tensor(
                        sq[:, dc, :], x_bf[:, dc, :], x_bf[:, dc, :],
                        mybir.AluOpType.mult,
                    )
                xs_ps = psP.tile([1, QLEN], DT, tag="proj")
                for dc in range(8):
                    nc.tensor.matmul(
                        xs_ps[:], ones_t[:], x_bf[:, dc, :],
                        start=(dc == 0), stop=(dc == 7),
                    )
                sq_ps = psP.tile([1, QLEN], DT, tag="proj")
                for dc in range(8):
                    nc.tensor.matmul(
                        sq_ps[:], ones_t[:], sq[:, dc, :],
                        start=(dc == 0), stop=(dc == 7),
                    )
                xs_sb = smp.tile([1, QLEN], DT, tag="xs")
                sq_sb = smp.tile([1, QLEN], DT, tag="sqs")
                nc.vector.tensor_copy(xs_sb[:], xs_ps[:])
                nc.vector.tensor_copy(sq_sb[:], sq_ps[:])
                XS = lnp.tile([128, QLEN], DT, tag="XS")
                SQ = lnp.tile([128, QLEN], DT, tag="SQ")
                nc.gpsimd.partition_broadcast(XS[:], xs_sb[:])
                nc.gpsimd.partition_broadcast(SQ[:], sq_sb[:])
                mu = lnp.tile([128, QLEN], DT, tag="mu")
                var = lnp.tile([128, QLEN], DT, tag="var")
                rstd = lnp.tile([128, QLEN], DT, tag="rstd")
                nc.vector.tensor_scalar_mul(mu[:], XS[:], 1.0 / D_MODEL)
                nc.vector.tensor_tensor(var[:], mu[:], mu[:], mybir.AluOpType.mult)
                nc.vector.tensor_scalar(
                    SQ[:], SQ[:], 1.0 / D_MODEL, EPS,
                    mybir.AluOpType.mult, mybir.AluOpType.add,
                )
                nc.vector.tensor_tensor(
                    var[:], SQ[:], var[:], mybir.AluOpType.subtract
                )
                nc.scalar.sqrt(var[:], var[:])
                nc.vector.reciprocal(rstd[:], var[:])
                for dc in range(8):
                    xc = lnp.tile([128, QLEN], DT, tag="xc")
                    nc.vector.tensor_tensor(
                        xc[:], x_bf[:, dc, :], mu[:], mybir.AluOpType.subtract
                    )
                    nc.vector.tensor_tensor(
                        xc[:], xc[:], rstd[:], mybir.AluOpType.mult
                    )
                    nc.vector.tensor_scalar(
                        x_bf[:, dc, :], xc[:], g_t[:, dc : dc + 1],
                        b_t[:, dc : dc + 1],
                        mybir.AluOpType.mult, mybir.AluOpType.add,
                    )

            def add_residual(ar_out, dc0, b2_t=None):
                """x_bf[dc0:dc0+4] += allgathered partial sums (+ b2)."""
                arr4 = ar_out.rearrange("r (c p) i -> r p c i", p=128)
                for c in range(4):
                    dc = dc0 + c
                    arr = arp.tile([128, QLEN], BF, tag="arr")
                    nc.sync.dma_start(arr[:], arr4[0, :, c, :])
                    nc.gpsimd.dma_start(
                        arr[:], arr4[1, :, c, :], accum_op=mybir.AluOpType.add
                    )
                    if b2_t is not None:
                        nc.vector.tensor_scalar(
                            arr[:], arr[:], b2_t[:, dc : dc + 1], None,
                            mybir.AluOpType.add,
                        )
                    nc.vector.tensor_tensor(
                        x_bf[:, dc, :], x_bf[:, dc, :], arr[:], mybir.AluOpType.add
                    )

            def project_exchange(matmul_half, b2_t=None):
                """8 dm-chunk projections in 2 collective halves; residual add."""
                for half in range(2):
                    ar_in = dramp.tile([D_MODEL // 2, QLEN], BF, tag=f"arin{half}")
                    ar_out = dramp.tile([2, D_MODEL // 2, QLEN], BF,
                                        tag=f"arout{half}")
                    ar4 = ar_in.rearrange("(c p) i -> p c i", p=128)
                    for c in range(4):
                        dmc = 4 * half + c
                        ops = psP.tile([128, QLEN], DT, tag="proj")
                        matmul_half(ops, dmc)
                        osb = arp.tile([128, QLEN], BF, tag="osb")
                        cp = nc.vector.tensor_copy if c % 2 else nc.scalar.copy
                        cp(osb[:], ops[:])
                        eng = nc.sync if c % 2 else nc.scalar
                        eng.dma_start(ar4[:, c, :], osb[:])
                    nc.gpsimd.collective_compute(
                        "AllGather", mybir.AluOpType.bypass,
                        replica_groups=PAIRS, ins=[ar_in.opt()],
                        outs=[ar_out.opt()],
                    )
                    add_residual(ar_out, 4 * half, b2_t=b2_t)

            for l in range(L):
                # ---- whole-layer loads ----
                wq_t = wtp.tile([128, 8, NDH], BF, tag="wq")
                wk_t = wtp.tile([128, 8, NDH], BF, tag="wk")
                wv_t = wtp.tile([128, 8, NDH], BF, tag="wv")
                nc.sync.dma_start(wq_t[:], wq_in[l])
                nc.sync.dma_start(wk_t[:], wk_in[l])
                nc.sync.dma_start(wv_t[:], wv_in[l])
                memT_t = actp.tile([128, 8, MLEN], BF, tag="memT")
                nc.sync.dma_start(memT_t[:], memT_in[l])
                b1_t = wtp.tile([128, 16], DT, tag="b1")
                nc.sync.dma_start(b1_t[:], b1_in[l])
                lnt = wtp.tile([128, 5, 8], DT, tag="lns")
                nc.sync.dma_start(lnt[:], lns_in[l])

                # ---- v projection: vv[p, kc, n] = v[128 kc + p, n], all heads ----
                vv = actp.tile([128, 8, NDH], BF, tag="vv")
                for kc in range(8):
                    vps = psP.tile([128, NDH], DT, tag="proj")
                    src = memT_t if kc < 4 else x_bf
                    ksl = slice(128 * (kc % 4), 128 * (kc % 4) + 128)
                    for dc in range(8):
                        nc.tensor.matmul(
                            vps[:], src[:, dc, ksl], wv_t[:, dc, :],
                            start=(dc == 0), stop=(dc == 7),
                        )
                    cp = nc.vector.tensor_copy if kc % 2 else nc.scalar.copy
                    cp(vv[:, kc, :], vps[:])

                pvT_all = actp.tile([128, 4, QLEN], BF, tag="pvT")

                # ---- attention, one 2-head pair (= one 128-row nd chunk) at a time
                for p in range(4):
                    nsl = slice(128 * p, 128 * p + 128)
                    # q^T + biases
                    qps = psP.tile([128, QLEN], DT, tag="proj")
                    for dc in range(8):
                        nc.tensor.matmul(
                            qps[:], wq_t[:, dc, nsl], x_bf[:, dc, :],
                            start=(dc == 0), stop=(dc == 7),
                        )
                    qbwT = prqp.tile([128, QLEN], BF, tag="qbw")
                    qbrT = prqp.tile([128, QLEN], BF, tag="qbr")
                    nc.scalar.add(qbwT[:], qps[:], bw_t[:, p : p + 1])
                    nc.scalar.add(qbrT[:], qps[:], br_t[:, p : p + 1])
                    # k^T chunk: kT[p2, j] with p2 = nd within chunk
                    kT = prqp.tile([128, KLEN], BF, tag="kT")
                    for kh in range(2):
                        kps = psP.tile([128, QLEN], DT, tag="proj")
                        src = memT_t if kh == 0 else x_bf
                        for dc in range(8):
                            nc.tensor.matmul(
                                kps[:], wk_t[:, dc, nsl], src[:, dc, :],
                                start=(dc == 0), stop=(dc == 7),
                            )
                        cp = nc.vector.tensor_copy if kh else nc.scalar.copy
                        cp(kT[:, 512 * kh : 512 * kh + 512], kps[:])
                    rkT = prqp.tile([128, KLEN], BF, tag="rkT")
                    nc.sync.dma_start(rkT[:], rkT_in[l, p])

                    # pre = (q+br)^T-chunk @ rkT per head, shingled-write to DRAM
                    scrs = []
                    for hh in range(2):
                        scr = dramp.tile([SCR_N], BF, tag=f"scr{hh}")
                        scrs.append(scr)
                    for ic in range(4):
                        pre_sb = prep.tile([128, 2, KLEN], BF, tag="pre_sb")
                        for hh in range(2):
                            base = 64 * hh
                            for jh in range(2):
                                pps = psSp.tile([128, 512], DT, tag="pre")
                                nc.tensor.matmul(
                                    pps[:],
                                    qbrT[base : base + 64, 128 * ic : 128 * ic + 128],
                                    rkT[base : base + 64, 512 * jh : 512 * jh + 512],
                                    start=True, stop=True,
                                )
                                cp = nc.vector.tensor_copy if jh else nc.scalar.copy
                                cp(pre_sb[:, hh, 512 * jh : 512 * jh + 512], pps[:])
                        for hh in range(2):
                            shingle = bass.AP(
                                scrs[hh].tensor,
                                scrs[hh].offset + 1 + 128 * ic * 1025,
                                [[1025, 128], [1, KLEN]],
                            )
                            eng = nc.sync if (ic + hh) % 2 == 0 else nc.scalar
                            eng.dma_start(shingle, pre_sb[:, hh, :])

                    # per head: transpose-read shifted BD^T, mask, AC, exp, PV
                    pv = psV.tile([128, QLEN], DT, tag="pv")
                    for hh in range(2):
                        base = 64 * hh
                        h2 = 2 * p + hh
                        scr = scrs[hh]
                        dense = scr[512 : 512 + QLEN * KLEN].rearrange(
                            "(i j) -> i j", j=KLEN
                        )
                        bd = hdp.tile([128, 8, QLEN], BF, tag=f"bd{hh}")
                        nc.scalar.dma_start_transpose(bd[:, 0:4, :], dense[:, 0:512])
                        nc.scalar.dma_start_transpose(bd[:, 4:8, :], dense[:, 512:])
                        # mask: keep i >= 128 jc + pj - 512 (only jc>=4 can mask)
                        for jc in range(4, 8):
                            w = 128 * (jc - 3)
                            nc.gpsimd.affine_select(
                                out=bd[:, jc, 0:w], in_=bd[:, jc, 0:w],
                                pattern=[[1, w]],
                                compare_op=mybir.AluOpType.is_ge,
                                fill=NEG, base=512 - 128 * jc,
                                channel_multiplier=-1,
                            )
                        if DEBUG and l == 0 and p == DBG_P and hh == DBG_HH:
                            nc.gpsimd.dma_start(dbg_bd[:], bd[:])
                        expT = hd1p.tile([128, 8, QLEN], BF, tag=f"expT{hh}")
                        for jc in range(8):
                            acp = psA.tile([128, QLEN], DT, tag="ac")
                            nc.tensor.matmul(
                                acp[:],
                                kT[base : base + 64, 128 * jc : 128 * jc + 128],
                                qbwT[base : base + 64, :],
                                start=True, stop=False,
                            )
                            # accumulate BD via identity matmul (keeps PE dense)
                            nc.tensor.matmul(
                                acp[:], id_t[:], bd[:, jc, :],
                                start=False, stop=True,
                            )
                            nc.scalar.activation(
                                expT[:, jc, :], acp[:],
                                mybir.ActivationFunctionType.Exp, scale=SCALE,
                            )
                        if DEBUG and l == 0 and p == DBG_P and hh == DBG_HH:
                            nc.gpsimd.dma_start(dbg_ex[:], expT[:])
                        # denominator: ones-matmul accumulation over j chunks
                        den_ps = psV.tile([1, QLEN], DT, tag="den")
                        for jc in range(8):
                            nc.tensor.matmul(
                                den_ps[:], ones_t[:], expT[:, jc, :],
                                start=(jc == 0), stop=(jc == 7),
                            )
                        den_sb = smp.tile([1, QLEN], DT, tag="rec")
                        nc.vector.tensor_copy(den_sb[:], den_ps[:])
                        denb = hd1p.tile([128, QLEN], DT, tag=f"recb{hh}")
                        nc.gpsimd.partition_broadcast(denb[:], den_sb[:])
                        nc.vector.reciprocal(denb[:], denb[:])
                        # PV: vecT_unnorm[d, i]; head A -> psum rows 0:64,
                        # head B -> rows 64:128 (col-group tiling)
                        for jc in range(8):
                            nc.tensor.matmul(
                                pv[base : base + 64, :],
                                vv[:, jc, 64 * h2 : 64 * h2 + 64],
                                expT[:, jc, :],
                                start=(jc == 0), stop=(jc == 7),
                                tile_position=(0, base),
                            )
                        nc.vector.tensor_tensor(
                            pvT_all[base : base + 64, p, :],
                            pv[base : base + 64, :], denb[base : base + 64, :],
                            mybir.AluOpType.mult,
                        )
                        if DEBUG and l == 0 and p == DBG_P and hh == 1:
                            pvsb = hd1p.tile([128, QLEN], DT, tag="dbgpv")
                            nc.vector.tensor_copy(pvsb[:], pv[:])
                            nc.gpsimd.dma_start(dbg_pvr[:], pvsb[:])
                            nc.gpsimd.dma_start(dbg_rb[:], recb[:])

                if DEBUG and l == 0:
                    nc.gpsimd.dma_start(dbg_pv[:], pvT_all[:])
                    nc.gpsimd.dma_start(dbg_kv[:], vv[:])
                # ---- attention out projection (transposed) + pairwise exchange
                wo_t = wtp.tile([128, 4, D_MODEL], BF, tag="wq")  # alias wq slot
                nc.sync.dma_start(wo_t[:], wo_in[l])

                def attn_out(ops, dmc):
                    for p in range(4):
                        nc.tensor.matmul(
                            ops[:],
                            wo_t[:, p, 128 * dmc : 128 * dmc + 128],
                            pvT_all[:, p, :],
                            start=(p == 0), stop=(p == 3),
                        )

                project_exchange(attn_out)
                layer_norm(lnt[:, 0, :], lnt[:, 1, :])
                if DEBUG and l == 0:
                    nc.gpsimd.dma_start(dbg_x1[:], x_bf[:])

                # ---- FFN ----
                hT = actp.tile([128, 16, QLEN], BF, tag="hT")
                for ic in range(16):
                    w1t = wstp.tile([128, 8, 128], BF, tag="w1t")
                    nc.sync.dma_start(w1t[:], w1_in[l, ic])
                    ps = psP.tile([128, QLEN], DT, tag="proj")
                    for dc in range(8):
                        nc.tensor.matmul(
                            ps[:], w1t[:, dc, :], x_bf[:, dc, :],
                            start=(dc == 0), stop=(dc == 7),
                        )
                    nc.scalar.activation(
                        hT[:, ic, :], ps[:], mybir.ActivationFunctionType.Relu,
                        bias=b1_t[:, ic : ic + 1],
                    )
                def ffn_out(ops, dmc):
                    w2t = wstp.tile([128, 16, 128], BF, tag="w2t")
                    nc.sync.dma_start(w2t[:], w2_in[l, dmc])
                    for ic in range(16):
                        nc.tensor.matmul(
                            ops[:], w2t[:, ic, :], hT[:, ic, :],
                            start=(ic == 0), stop=(ic == 15),
                        )

                project_exchange(ffn_out, b2_t=lnt[:, 4, :])
                layer_norm(lnt[:, 2, :], lnt[:, 3, :])

            # ---- final hidden out + unembed partials ----
            nc.sync.dma_start(xout[:], x_bf[:])
            for vt in range(NVT):
                # double-buffer embT tiles in the dead wk/wv weight slots
                et = wtp.tile([128, 8, VT], BF, tag=("wk" if vt % 2 == 0 else "wv"))
                nc.sync.dma_start(et[:], embT_in[vt])
                for qc in range(4):
                    lps = psP.tile([128, QLEN], DT, tag="proj")
                    for dc in range(8):
                        nc.tensor.matmul(
                            lps[:, 0:VT],
                            x_bf[:, dc, 128 * qc : 128 * qc + 128],
                            et[:, dc, :],
                            start=(dc == 0), stop=(dc == 7),
                        )
                    nc.vector.tensor_reduce(
                        lmax_sb[:, qc, vt : vt + 1], lps[:, 0:VT],
                        mybir.AxisListType.X, mybir.AluOpType.max,
                    )
                    negm = smp.tile([128, 1], DT, tag="negm")
                    nc.vector.tensor_scalar_mul(
                        negm[:], lmax_sb[:, qc, vt : vt + 1], -1.0
                    )
                    lsc = smp.tile([128, VT], BF, tag="lsc")
                    nc.scalar.activation(
                        lsc[:], lps[:, 0:VT], mybir.ActivationFunctionType.Exp,
                        bias=negm[:], accum_out=lsum_sb[:, qc, vt : vt + 1],
                    )
            nc.sync.dma_start(lmax_out[:], lmax_sb[:])
            nc.sync.dma_start(lsum_out[:], lsum_sb[:])

    nc.compile()
    return nc


def _get_nc():
    if "nc" not in _CACHE:
        _CACHE["nc"] = _build()
    return _CACHE["nc"]


def _make_pos():
    pos_seq = np.arange(KLEN - 1, -1, -1, dtype=F32)
    inv_freq = 1.0 / (10000.0 ** (np.arange(0, D_MODEL, 2, dtype=F32) / D_MODEL))
    sin_inp = np.outer(pos_seq, inv_freq).astype(F32)
    return np.concatenate([np.sin(sin_inp), np.cos(sin_inp)], -1).astype(F32)


def _prep_inputs(data, memory, emb, Wq, Wkv, Wr, Wo, ffW1, ffb1, ffW2, ffb2,
                 ln1_g, ln1_b, ln2_g, ln2_b, bias_w, bias_r):
    # honor a reduced layer count (debugging)
    memory, Wq, Wkv, Wr, Wo = memory[:L], Wq[:L], Wkv[:L], Wr[:L], Wo[:L]
    ffW1, ffb1, ffW2, ffb2 = ffW1[:L], ffb1[:L], ffW2[:L], ffb2[:L]
    ln1_g, ln1_b, ln2_g, ln2_b = ln1_g[:L], ln1_b[:L], ln2_g[:L], ln2_b[:L]
    pos = _make_pos()                                  # [KLEN, D_MODEL]
    rk = np.einsum("kd,ldn->lkn", pos, Wr.astype(F32))  # [L, KLEN, 2*NDH]
    embT = np.ascontiguousarray(emb.T).astype(BF16)    # [D_MODEL, VOCAB]
    bwf = bias_w.reshape(-1).astype(F32)
    brf = bias_r.reshape(-1).astype(F32)

    def chunk(w, c):
        # [L, D, N] -> [L, 128, c, N] with row index = 128*ci + p
        L_, D_, N_ = w.shape
        return np.ascontiguousarray(
            w.reshape(L_, c, 128, N_).transpose(0, 2, 1, 3)).astype(BF16)

    def percol(v):
        # [L, D] -> [L, 128, D//128] per-partition layout
        return np.ascontiguousarray(
            v.reshape(L, -1, 128).transpose(0, 2, 1)).astype(F32)

    in_maps = []
    for c in range(NCORES):
        b, h = c // 2, c % 2
        nds = slice(NDH * h, NDH * h + NDH)
        dis = slice(DIH * h, DIH * h + DIH)
        rkTh = np.ascontiguousarray(
            rk[:, :, nds].transpose(0, 2, 1).reshape(L, 4, 128, KLEN)
        ).astype(BF16)
        memTb = np.ascontiguousarray(memory[:, b].transpose(0, 2, 1))  # [L,1024,512]
        embTh = embT[:, VSH * h : VSH * h + VSH]                       # [1024, VSH]
        embT4 = np.ascontiguousarray(
            embTh.reshape(8, 128, NVT, VT).transpose(2, 1, 0, 3))      # [NVT,128,8,VT]
        x0 = emb[np.asarray(data[b])].astype(F32)                      # [512, 1024]
        x0T = np.ascontiguousarray(
            x0.T.reshape(8, 128, QLEN).transpose(1, 0, 2)).astype(BF16)
        w1h = ffW1[:, :, dis]                                          # [L, 1024, 2048]
        w1g = np.ascontiguousarray(
            w1h.reshape(L, 8, 128, 16, 128).transpose(0, 3, 2, 1, 4)).astype(BF16)
        w2h = ffW2[:, dis, :]                                          # [L, 2048, 1024]
        w2g = np.ascontiguousarray(
            w2h.reshape(L, 16, 128, 8, 128).transpose(0, 3, 2, 1, 4)).astype(BF16)
        in_maps.append({
            "x0": x0T,
            "memT": chunk(memTb, 8),
            "wq": chunk(Wq[:, :, nds], 8),
            "wk": chunk(Wkv[:, :, nds], 8),
            "wv": chunk(Wkv[:, :, D_MODEL + NDH * h : D_MODEL + NDH * h + NDH], 8),
            "rkT": rkTh,
            "wo": chunk(Wo[:, nds, :], 4),
            "w1": w1g,
            "w2": w2g,
            "b1": np.ascontiguousarray(
                ffb1[:, dis].reshape(L, 16, 128).transpose(0, 2, 1)).astype(F32),
            "lns": np.ascontiguousarray(np.stack(
                [percol(np.asarray(v)) for v in
                 (ln1_g, ln1_b, ln2_g, ln2_b, ffb2)], axis=2)),
            "bw": np.ascontiguousarray(bwf[nds].reshape(4, 128).T),
            "br": np.ascontiguousarray(brf[nds].reshape(4, 128).T),
            "embT": embT4,
        })
    return in_maps


def _combine(results, target, emb):
    nll = np.zeros((BSZ, QLEN), dtype=np.float64)
    for b in range(BSZ):
        r0, r1 = results[2 * b], results[2 * b + 1]
        lm = np.concatenate([r0["lmax"], r1["lmax"]], axis=-1).astype(np.float64)
        ls = np.concatenate([r0["lsum"], r1["lsum"]], axis=-1).astype(np.float64)
        M = lm.max(-1)                                   # [128, 4]
        Z = (ls * np.exp(lm - M[..., None])).sum(-1)     # [128, 4]
        logZ = (M + np.log(Z)).transpose(1, 0).reshape(QLEN)  # i = 128*qc + p
        # xout[p, dc, i] = x[i, 128 dc + p]
        xf = r0["xout"].astype(np.float64).transpose(2, 1, 0).reshape(QLEN, D_MODEL)
        et = emb[np.asarray(target[b])].astype(BF16).astype(np.float64)
        tgt = (xf * et).sum(-1)
        nll[b] = logZ - tgt
    return nll.astype(F32).reshape(-1).reshape(QLEN, BSZ)


def _prep_all(inputs):
    return _prep_inputs(
        np.asarray(inputs["data"]), np.asarray(inputs["memory"], dtype=F32),
        np.asarray(inputs["emb"], dtype=F32),
        np.asarray(inputs["Wq"], dtype=F32), np.asarray(inputs["Wkv"], dtype=F32),
        np.asarray(inputs["Wr"], dtype=F32), np.asarray(inputs["Wo"], dtype=F32),
        np.asarray(inputs["ffW1"], dtype=F32), np.asarray(inputs["ffb1"], dtype=F32),
        np.asarray(inputs["ffW2"], dtype=F32), np.asarray(inputs["ffb2"], dtype=F32),
        np.asarray(inputs["ln1_g"], dtype=F32), np.asarray(inputs["ln1_b"], dtype=F32),
        np.asarray(inputs["ln2_g"], dtype=F32), np.asarray(inputs["ln2_b"], dtype=F32),
        np.asarray(inputs["bias_w"], dtype=F32), np.asarray(inputs["bias_r"], dtype=F32),
    )


def kernel(**inputs):
    nc = _get_nc()
    target = np.asarray(inputs["target"])
    emb = np.asarray(inputs["emb"], dtype=F32)
    in_maps = _prep_all(inputs)
    res = run_bass_kernel_spmd(nc, in_maps, core_ids=list(range(NCORES)))
    return _combine(res.results, target, emb)


# revision 42
# speedup vs baseline: 1.1222x; 1.0740x over previous
"""MemTransformerLM (Transformer-XL) forward pass on 8 TRN2 NeuronCores.

Sharding: core c handles batch b = c//2 and tensor-parallel half h = c%2
(heads 8h..8h+8 of 16; FFN inner columns 2048h..2048h+2048 of 4096; vocab
16000h.. for the final logsumexp). Pairwise AllGather + local add after the
attention output projection and after FFN2.

Layout: the residual stream lives TRANSPOSED in SBUF as x[p, dc, i] =
x[i, 128*dc+p] (d on partitions), bf16. All projections consume it directly
as the matmul moving operand; attention scores are computed transposed
(scoreT[j, i], klen on partitions) so softmax probs feed PV without any
transpose. Softmax is unnormalized: exp(score*scale) accumulates through PV
and the out-projection input is scaled by 1/den per head beforehand.

rel_shift: pre[i, jj] = (q+br)_i . rk_jj is written to DRAM with SHINGLED
rows (row i at element offset i*1025 + 1). Then the plain dense [512, 1024]
view at element offset 512 satisfies dense[i, j] = pre[i, j + 511 - i] =
shifted BD, and a transpose-DMA of that view yields BD^T[j, i] directly.
Positions j > i + 512 read garbage; affine_select fills them with -1e30
(the causal mask), so exp gives exactly 0.

LayerNorm in transposed layout: token sums via ones-matmul into PSUM[1, i],
broadcast back across partitions, elementwise apply with per-partition g/b.

All matmuls bf16 with fp32 PSUM accumulation.
"""

import numpy as np
import ml_dtypes

import concourse.bass as bass
import concourse.mybir as mybir
import concourse.tile as tile
from concourse import bacc
from concourse.bass_utils import run_bass_kernel_spmd

# Model dims (hardcoded per problem spec)
L = 6
D_MODEL = 1024
D_HEAD = 64
D_INNER = 4096
BSZ = 4
QLEN = 512
MLEN = 512
KLEN = MLEN + QLEN
VOCAB = 32000
SCALE = 1.0 / (D_HEAD ** 0.5)
EPS = 1e-5
NEG = -1e30

NCORES = 8
NDH = 512          # nd per core (8 heads x 64)
DIH = 2048         # ffn inner per core
VSH = VOCAB // 2   # vocab per core (split across the pair)
VT = 400           # vocab tile width
NVT = VSH // VT    # 40

# shingled scratch: row i of pre written at element offset 1 + i*1025
SCR_N = 512 * 1025 + 1024 + 64

DT = mybir.dt.float32
BF = mybir.dt.bfloat16
F32 = np.float32
BF16 = ml_dtypes.bfloat16

PAIRS = [[0, 1], [2, 3], [4, 5], [6, 7]]

DEBUG = False  # add layer-0 intermediate dumps
DBG_P = 0   # which head-pair the bd/ex/pvr taps target
DBG_HH = 0

_CACHE: dict = {}


def _build():
    nc = bacc.Bacc("TRN2", target_bir_lowering=False, debug=False, num_devices=NCORES)

    # ---- I/O ----
    x0_in = nc.dram_tensor("x0", [128, 8, QLEN], BF, kind="ExternalInput")
    memT_in = nc.dram_tensor("memT", [L, 128, 8, MLEN], BF, kind="ExternalInput")
    wq_in = nc.dram_tensor("wq", [L, 128, 8, NDH], BF, kind="ExternalInput")
    wk_in = nc.dram_tensor("wk", [L, 128, 8, NDH], BF, kind="ExternalInput")
    wv_in = nc.dram_tensor("wv", [L, 128, 8, NDH], BF, kind="ExternalInput")
    rkT_in = nc.dram_tensor("rkT", [L, 4, 128, KLEN], BF, kind="ExternalInput")
    wo_in = nc.dram_tensor("wo", [L, 128, 4, D_MODEL], BF, kind="ExternalInput")
    # w1 regrouped per inner-chunk: [L, 16, 128, 8, 128]
    w1_in = nc.dram_tensor("w1", [L, 16, 128, 8, 128], BF, kind="ExternalInput")
    # w2 regrouped per dmodel-chunk: [L, 8, 128, 16, 128]
    w2_in = nc.dram_tensor("w2", [L, 8, 128, 16, 128], BF, kind="ExternalInput")
    b1_in = nc.dram_tensor("b1", [L, 128, 16], DT, kind="ExternalInput")
    # packed per-partition params: [g1, bg1, g2, bg2, b2] along dim 2
    lns_in = nc.dram_tensor("lns", [L, 128, 5, 8], DT, kind="ExternalInput")
    bw_in = nc.dram_tensor("bw", [128, 4], DT, kind="ExternalInput")
    br_in = nc.dram_tensor("br", [128, 4], DT, kind="ExternalInput")
    embT_in = nc.dram_tensor("embT", [NVT, 128, 8, VT], BF, kind="ExternalInput")

    xout = nc.dram_tensor("xout", [128, 8, QLEN], BF, kind="ExternalOutput")
    lmax_out = nc.dram_tensor("lmax", [128, 4, NVT], DT, kind="ExternalOutput")
    lsum_out = nc.dram_tensor("lsum", [128, 4, NVT], DT, kind="ExternalOutput")
    if DEBUG:
        dbg_pv = nc.dram_tensor("dbg_pv", [128, 4, QLEN], DT, kind="ExternalOutput")
        dbg_bd = nc.dram_tensor("dbg_bd", [128, 8, QLEN], DT, kind="ExternalOutput")
        dbg_ex = nc.dram_tensor("dbg_ex", [128, 8, QLEN], DT, kind="ExternalOutput")
        dbg_x1 = nc.dram_tensor("dbg_x1", [128, 8, QLEN], DT, kind="ExternalOutput")
        dbg_kv = nc.dram_tensor("dbg_kv", [128, 8, QLEN], DT, kind="ExternalOutput")
        dbg_rb = nc.dram_tensor("dbg_rb", [128, QLEN], DT, kind="ExternalOutput")
        dbg_pvr = nc.dram_tensor("dbg_pvr", [128, QLEN], DT, kind="ExternalOutput")

    from contextlib import ExitStack
    with tile.TileContext(nc) as tc:
        with ExitStack() as stack:
            ep = stack.enter_context
            constp = ep(tc.tile_pool(name="const", bufs=1))
            resp = ep(tc.tile_pool(name="res", bufs=1))
            wtp = ep(tc.tile_pool(name="wts", bufs=1))
            wstp = ep(tc.tile_pool(name="wst", bufs=3))   # streamed weight tiles
            actp = ep(tc.tile_pool(name="act", bufs=1))
            prqp = ep(tc.tile_pool(name="prq", bufs=4))   # per-pair q/k/rk tiles
            hdp = ep(tc.tile_pool(name="hd", bufs=2))     # per-head bd tiles
            hd1p = ep(tc.tile_pool(name="hd1", bufs=1))   # exp/esum/recb tiles
            prep = ep(tc.tile_pool(name="pre2", bufs=2))  # pre SBUF staging
            lnp = ep(tc.tile_pool(name="lnp", bufs=1))    # LN scratch
            smp = ep(tc.tile_pool(name="small", bufs=2))
            arp = ep(tc.tile_pool(name="arp", bufs=2))    # collective readback
            psA = ep(tc.tile_pool(name="ps_ac", bufs=3, space="PSUM"))
            psSp = ep(tc.tile_pool(name="ps_pre", bufs=2, space="PSUM"))
            psV = ep(tc.tile_pool(name="ps_pv", bufs=1, space="PSUM"))
            psP = ep(tc.tile_pool(name="ps_proj", bufs=2, space="PSUM"))
            dramp = ep(tc.tile_pool(name="dram", bufs=2, space="DRAM"))
            bw_t = constp.tile([128, 4], DT)
            br_t = constp.tile([128, 4], DT)
            ones_t = constp.tile([128, 1], BF)
            nc.sync.dma_start(bw_t[:], bw_in[:])
            nc.sync.dma_start(br_t[:], br_in[:])
            nc.vector.memset(ones_t[:], 1.0)
            id_t = constp.tile([128, 128], BF)
            nc.vector.memset(id_t[:], 1.0)
            nc.gpsimd.affine_select(
                out=id_t[:], in_=id_t[:], pattern=[[-1, 128]],
                compare_op=mybir.AluOpType.is_equal, fill=0.0,
                base=0, channel_multiplier=1,
            )

            # residual stream, bf16, transposed: x[p, dc, i] = x[i, 128 dc + p]
            x_bf = resp.tile([128, 8, QLEN], BF)
            nc.sync.dma_start(x_bf[:], x0_in[:])
            lmax_sb = resp.tile([128, 4, NVT], DT)
            lsum_sb = resp.tile([128, 4, NVT], DT)

            def layer_norm(g_t, b_t):
                """LN over d (partitions x 8 chunks) of x_bf, in place."""
                sq = actp.tile([128, 8, QLEN], BF, tag="hT")  # alias hT slot
                for dc in range(8):
                    nc.vector.tensor_tensor(
                        sq[:, dc, :], x_bf[:, dc, :], x_bf[:, dc, :],
                        mybir.AluOpType.mult,
                    )
                xs_ps = psP.tile([1, QLEN], DT, tag="proj")
                for dc in range(8):
                    nc.tensor.matmul(
                        xs_ps[:], ones_t[:], x_bf[:, dc, :],
                        start=(dc == 0), stop=(dc == 7),
                    )
                sq_ps = psP.tile([1, QLEN], DT, tag="proj")
                for dc in range(8):
                    nc.tensor.matmul(
                        sq_ps[:], ones_t[:], sq[:, dc, :],
                        start=(dc == 0), stop=(dc == 7),
                    )
                xs_sb = smp.tile([1, QLEN], DT, tag="xs")
                sq_sb = smp.tile([1, QLEN], DT, tag="sqs")
                nc.vector.tensor_copy(xs_sb[:], xs_ps[:])
                nc.vector.tensor_copy(sq_sb[:], sq_ps[:])
                XS = lnp.tile([128, QLEN], DT, tag="XS")
                SQ = lnp.tile([128, QLEN], DT, tag="SQ")
                nc.gpsimd.partition_broadcast(XS[:], xs_sb[:])
                nc.gpsimd.partition_broadcast(SQ[:], sq_sb[:])
                mu = lnp.tile([128, QLEN], DT, tag="mu")
                var = lnp.tile([128, QLEN], DT, tag="var")
                rstd = lnp.tile([128, QLEN], DT, tag="rstd")
                nc.vector.tensor_scalar_mul(mu[:], XS[:], 1.0 / D_MODEL)
                nc.vector.tensor_tensor(var[:], mu[:], mu[:], mybir.AluOpType.mult)
                nc.vector.tensor_scalar(
                    SQ[:], SQ[:], 1.0 / D_MODEL, EPS,
                    mybir.AluOpType.mult, mybir.AluOpType.add,
                )
                nc.vector.tensor_tensor(
                    var[:], SQ[:], var[:], mybir.AluOpType.subtract
                )
                nc.scalar.sqrt(var[:], var[:])
                nc.vector.reciprocal(rstd[:], var[:])
                for dc in range(8):
                    xc = lnp.tile([128, QLEN], DT, tag="xc")
                    nc.vector.tensor_tensor(
                        xc[:], x_bf[:, dc, :], mu[:], mybir.AluOpType.subtract
                    )
                    nc.vector.tensor_tensor(
                        xc[:], xc[:], rstd[:], mybir.AluOpType.mult
                    )
                    nc.vector.tensor_scalar(
                        x_bf[:, dc, :], xc[:], g_t[:, dc : dc + 1],
                        b_t[:, dc : dc + 1],
                        mybir.AluOpType.mult, mybir.AluOpType.add,
                    )

            def add_residual(ar_out, dc0, b2_t=None):
                """x_bf[dc0:dc0+4] += allgathered partial sums (+ b2)."""
                arr4 = ar_out.rearrange("r (c p) i -> r p c i", p=128)
                for c in range(4):
                    dc = dc0 + c
                    arr = arp.tile([128, QLEN], BF, tag="arr")
                    nc.sync.dma_start(arr[:], arr4[0, :, c, :])
                    nc.gpsimd.dma_start(
                        arr[:], arr4[1, :, c, :], accum_op=mybir.AluOpType.add
                    )
                    if b2_t is not None:
                        nc.vector.tensor_scalar(
                            arr[:], arr[:], b2_t[:, dc : dc + 1], None,
                            mybir.AluOpType.add,
                        )
                    nc.vector.tensor_tensor(
                        x_bf[:, dc, :], x_bf[:, dc, :], arr[:], mybir.AluOpType.add
                    )

            def project_exchange(matmul_half, b2_t=None):
                """8 dm-chunk projections in 2 collective halves; residual add."""
                for half in range(2):
                    ar_in = dramp.tile([D_MODEL // 2, QLEN], BF, tag=f"arin{half}")
                    ar_out = dramp.tile([2, D_MODEL // 2, QLEN], BF,
                                        tag=f"arout{half}")
                    ar4 = ar_in.rearrange("(c p) i -> p c i", p=128)
                    for c in range(4):
                        dmc = 4 * half + c
                        ops = psP.tile([128, QLEN], DT, tag="proj")
                        matmul_half(ops, dmc)
                        osb = arp.tile([128, QLEN], BF, tag="osb")
                        cp = nc.vector.tensor_copy if c % 2 else nc.scalar.copy
                        cp(osb[:], ops[:])
                        eng = nc.sync if c % 2 else nc.scalar
                        eng.dma_start(ar4[:, c, :], osb[:])
                    nc.gpsimd.collective_compute(
                        "AllGather", mybir.AluOpType.bypass,
                        replica_groups=PAIRS, ins=[ar_in.opt()],
                        outs=[ar_out.opt()],
                    )
                    add_residual(ar_out, 4 * half, b2_t=b2_t)

            for l in range(L):
                # ---- whole-layer loads ----
                wq_t = wtp.tile([128, 8, NDH], BF, tag="wq")
                wk_t = wtp.tile([128, 8, NDH], BF, tag="wk")
                wv_t = wtp.tile([128, 8, NDH], BF, tag="wv")
                nc.sync.dma_start(wq_t[:], wq_in[l])
                nc.sync.dma_start(wk_t[:], wk_in[l])
                nc.sync.dma_start(wv_t[:], wv_in[l])
                memT_t = actp.tile([128, 8, MLEN], BF, tag="memT")
                nc.sync.dma_start(memT_t[:], memT_in[l])
                b1_t = wtp.tile([128, 16], DT, tag="b1")
                nc.sync.dma_start(b1_t[:], b1_in[l])
                lnt = wtp.tile([128, 5, 8], DT, tag="lns")
                nc.sync.dma_start(lnt[:], lns_in[l])

                # ---- v projection (mem half first: x-independent) ----
                vv = actp.tile([128, 8, NDH], BF, tag="vv")

                def v_proj(kc):
                    vps = psP.tile([128, NDH], DT, tag="proj")
                    src = memT_t if kc < 4 else x_bf
                    ksl = slice(128 * (kc % 4), 128 * (kc % 4) + 128)
                    for dc in range(8):
                        nc.tensor.matmul(
                            vps[:], src[:, dc, ksl], wv_t[:, dc, :],
                            start=(dc == 0), stop=(dc == 7),
                        )
                    cp = nc.vector.tensor_copy if kc % 2 else nc.scalar.copy
                    cp(vv[:, kc, :], vps[:])

                for kc in range(4):
                    v_proj(kc)
                # k mem half (x-independent; fills the layer-boundary bubble)
                kTs = []
                for p in range(4):
                    nsl = slice(128 * p, 128 * p + 128)
                    kT = prqp.tile([128, KLEN], BF, tag="kT")
                    kTs.append(kT)
                    kps = psP.tile([128, QLEN], DT, tag="proj")
                    for dc in range(8):
                        nc.tensor.matmul(
                            kps[:], wk_t[:, dc, nsl], memT_t[:, dc, :],
                            start=(dc == 0), stop=(dc == 7),
                        )
                    cp = nc.vector.tensor_copy if p % 2 else nc.scalar.copy
                    cp(kT[:, 0:512], kps[:])
                for kc in range(4, 8):
                    v_proj(kc)

                pvT_all = actp.tile([128, 4, QLEN], BF, tag="pvT")
                st = {}

                def phase_a(p):
                    """q/k-x/pre/shear for pair p (all DMAs in flight early)."""
                    nsl = slice(128 * p, 128 * p + 128)
                    kT = kTs[p]
                    qps = psP.tile([128, QLEN], DT, tag="proj")
                    for dc in range(8):
                        nc.tensor.matmul(
                            qps[:], wq_t[:, dc, nsl], x_bf[:, dc, :],
                            start=(dc == 0), stop=(dc == 7),
                        )
                    qbwT = prqp.tile([128, QLEN], BF, tag="qbw")
                    qbrT = prqp.tile([128, QLEN], BF, tag="qbr")
                    nc.scalar.add(qbwT[:], qps[:], bw_t[:, p : p + 1])
                    nc.scalar.add(qbrT[:], qps[:], br_t[:, p : p + 1])
                    kps = psP.tile([128, QLEN], DT, tag="proj")
                    for dc in range(8):
                        nc.tensor.matmul(
                            kps[:], wk_t[:, dc, nsl], x_bf[:, dc, :],
                            start=(dc == 0), stop=(dc == 7),
                        )
                    cp = nc.vector.tensor_copy if p % 2 else nc.scalar.copy
                    cp(kT[:, 512:], kps[:])
                    rkT = prqp.tile([128, KLEN], BF, tag="rkT")
                    nc.sync.dma_start(rkT[:], rkT_in[l, p])

                    scrs = [dramp.tile([SCR_N], BF, tag=f"scr{hh}", name=f"scr{hh}")
                            for hh in range(2)]
                    for ic in range(4):
                        pre_sb = prep.tile([128, 2, KLEN], BF, tag="pre_sb")
                        for hh in range(2):
                            base = 64 * hh
                            for jh in range(2):
                                pps = psSp.tile([128, 512], DT, tag="pre")
                                nc.tensor.matmul(
                                    pps[:],
                                    qbrT[base : base + 64, 128 * ic : 128 * ic + 128],
                                    rkT[base : base + 64, 512 * jh : 512 * jh + 512],
                                    start=True, stop=True,
                                )
                                cp = nc.vector.tensor_copy if jh else nc.scalar.copy
                                cp(pre_sb[:, hh, 512 * jh : 512 * jh + 512], pps[:])
                        for hh in range(2):
                            shingle = bass.AP(
                                scrs[hh].tensor,
                                scrs[hh].offset + 1 + 128 * ic * 1025,
                                [[1025, 128], [1, KLEN]],
                            )
                            eng = nc.sync if (ic + hh) % 2 == 0 else nc.scalar
                            eng.dma_start(shingle, pre_sb[:, hh, :])
                    bds = []
                    for hh in range(2):
                        scr = scrs[hh]
                        dense = scr[512 : 512 + QLEN * KLEN].rearrange(
                            "(i j) -> i j", j=KLEN
                        )
                        bd = hdp.tile([128, 8, QLEN], BF, tag=f"bd{hh}")
                        bds.append(bd)
                        nc.scalar.dma_start_transpose(bd[:, 0:4, :], dense[:, 0:512])
                        nc.scalar.dma_start_transpose(bd[:, 4:8, :], dense[:, 512:])
                        # mask: keep i >= 128 jc + pj - 512 (only jc>=4 can mask)
                        for jc in range(4, 8):
                            w = 128 * (jc - 3)
                            nc.gpsimd.affine_select(
                                out=bd[:, jc, 0:w], in_=bd[:, jc, 0:w],
                                pattern=[[1, w]],
                                compare_op=mybir.AluOpType.is_ge,
                                fill=NEG, base=512 - 128 * jc,
                                channel_multiplier=-1,
                            )
                    st[p] = (qbwT, bds)

                def phase_b(p):
                    """AC bursts then den/PV/normalize for pair p."""
                    qbwT, bds = st.pop(p)
                    kT = kTs[p]
                    expTs = []
                    for hh in range(2):
                        base = 64 * hh
                        bd = bds[hh]
                        if DEBUG and l == 0 and p == DBG_P and hh == DBG_HH:
                            nc.gpsimd.dma_start(dbg_bd[:], bd[:])
                        expT = hd1p.tile([128, 8, QLEN], BF, tag=f"expT{hh}")
                        expTs.append(expT)
                        for jc in range(8):
                            acp = psA.tile([128, QLEN], DT, tag="ac")
                            nc.tensor.matmul(
                                acp[:],
                                kT[base : base + 64, 128 * jc : 128 * jc + 128],
                                qbwT[base : base + 64, :],
                                start=True, stop=False,
                            )
                            nc.tensor.matmul(
                                acp[:], id_t[:], bd[:, jc, :],
                                start=False, stop=True,
                            )
                            nc.scalar.activation(
                                expT[:, jc, :], acp[:],
                                mybir.ActivationFunctionType.Exp, scale=SCALE,
                            )
                        if DEBUG and l == 0 and p == DBG_P and hh == DBG_HH:
                            nc.gpsimd.dma_start(dbg_ex[:], expT[:])
                    pv = psV.tile([128, QLEN], DT, tag="pv")
                    for hh in range(2):
                        base = 64 * hh
                        h2 = 2 * p + hh
                        expT = expTs[hh]
                        den_ps = psSp.tile([1, QLEN], DT, tag="pre")
                        for jc in range(8):
                            nc.tensor.matmul(
                                den_ps[:], ones_t[:], expT[:, jc, :],
                                start=(jc == 0), stop=(jc == 7),
                            )
                        den_sb = smp.tile([1, QLEN], DT, tag="rec")
                        nc.vector.tensor_copy(den_sb[:], den_ps[:])
                        denb = hd1p.tile([128, QLEN], DT, tag=f"recb{hh}")
                        nc.gpsimd.partition_broadcast(denb[:], den_sb[:])
                        nc.vector.reciprocal(denb[:], denb[:])
                        for jc in range(8):
                            nc.tensor.matmul(
                                pv[base : base + 64, :],
                                vv[:, jc, 64 * h2 : 64 * h2 + 64],
                                expT[:, jc, :],
                                start=(jc == 0), stop=(jc == 7),
                                tile_position=(0, base),
                            )
                        if DEBUG and l == 0 and p == DBG_P and hh == 1:
                            pvsb = hd1p.tile([128, QLEN], DT, tag="dbgpv")
                            nc.vector.tensor_copy(pvsb[:], pv[:])
                            nc.gpsimd.dma_start(dbg_pvr[:], pvsb[:])
                            nc.gpsimd.dma_start(dbg_rb[:], denb[:])
                        nc.vector.tensor_tensor(
                            pvT_all[base : base + 64, p, :],
                            pv[base : base + 64, :], denb[base : base + 64, :],
                            mybir.AluOpType.mult,
                        )

                # software pipeline: shear DMAs run 1-2 pairs ahead of compute
                phase_a(0)
                phase_a(1)
                phase_b(0)
                phase_a(2)
                phase_b(1)
                phase_a(3)
                phase_b(2)
                phase_b(3)

                if DEBUG and l == 0:
                    nc.gpsimd.dma_start(dbg_pv[:], pvT_all[:])
                    nc.gpsimd.dma_start(dbg_kv[:], vv[:])
                # ---- attention out projection (transposed) + pairwise exchange
                wo_t = wtp.tile([128, 4, D_MODEL], BF, tag="wq")  # alias wq slot
                nc.sync.dma_start(wo_t[:], wo_in[l])

                def attn_out(ops, dmc):
                    for p in range(4):
                        nc.tensor.matmul(
                            ops[:],
                            wo_t[:, p, 128 * dmc : 128 * dmc + 128],
                            pvT_all[:, p, :],
                            start=(p == 0), stop=(p == 3),
                        )

                project_exchange(attn_out)
                layer_norm(lnt[:, 0, :], lnt[:, 1, :])
                if DEBUG and l == 0:
                    nc.gpsimd.dma_start(dbg_x1[:], x_bf[:])

                # ---- FFN ----
                hT = actp.tile([128, 16, QLEN], BF, tag="hT")
                for ic in range(16):
                    w1t = wstp.tile([128, 8, 128], BF, tag="w1t")
                    nc.sync.dma_start(w1t[:], w1_in[l, ic])
                    ps = psP.tile([128, QLEN], DT, tag="proj")
                    for dc in range(8):
                        nc.tensor.matmul(
                            ps[:], w1t[:, dc, :], x_bf[:, dc, :],
                            start=(dc == 0), stop=(dc == 7),
                        )
                    nc.scalar.activation(
                        hT[:, ic, :], ps[:], mybir.ActivationFunctionType.Relu,
                        bias=b1_t[:, ic : ic + 1],
                    )
                def ffn_out(ops, dmc):
                    w2t = wstp.tile([128, 16, 128], BF, tag="w2t")
                    nc.sync.dma_start(w2t[:], w2_in[l, dmc])
                    for ic in range(16):
                        nc.tensor.matmul(
                            ops[:], w2t[:, ic, :], hT[:, ic, :],
                            start=(ic == 0), stop=(ic == 15),
                        )

                project_exchange(ffn_out, b2_t=lnt[:, 4, :])
                layer_norm(lnt[:, 2, :], lnt[:, 3, :])

            # ---- final hidden out + unembed partials ----
            nc.sync.dma_start(xout[:], x_bf[:])
            for vt in range(NVT):
                # double-buffer embT tiles in the dead wk/wv weight slots
                et = wtp.tile([128, 8, VT], BF, tag=("wk" if vt % 2 == 0 else "wv"))
                nc.sync.dma_start(et[:], embT_in[vt])
                for qc in range(4):
                    lps = psP.tile([128, QLEN], DT, tag="proj")
                    for dc in range(8):
                        nc.tensor.matmul(
                            lps[:, 0:VT],
                            x_bf[:, dc, 128 * qc : 128 * qc + 128],
                            et[:, dc, :],
                            start=(dc == 0), stop=(dc == 7),
                        )
                    nc.vector.tensor_reduce(
                        lmax_sb[:, qc, vt : vt + 1], lps[:, 0:VT],
                        mybir.AxisListType.X, mybir.AluOpType.max,
                    )
                    negm = smp.tile([128, 1], DT, tag="negm")
                    nc.vector.tensor_scalar_mul(
                        negm[:], lmax_sb[:, qc, vt : vt + 1], -1.0
                    )
                    lsc = smp.tile([128, VT], BF, tag="lsc")
                    nc.scalar.activation(
                        lsc[:], lps[:, 0:VT], mybir.ActivationFunctionType.Exp,
                        bias=negm[:], accum_out=lsum_sb[:, qc, vt : vt + 1],
                    )
            nc.sync.dma_start(lmax_out[:], lmax_sb[:])
            nc.sync.dma_start(lsum_out[:], lsum_sb[:])

    nc.compile()
    return nc


def _get_nc():
    if "nc" not in _CACHE:
        _CACHE["nc"] = _build()
    return _CACHE["nc"]


def _make_pos():
    pos_seq = np.arange(KLEN - 1, -1, -1, dtype=F32)
    inv_freq = 1.0 / (10000.0 ** (np.arange(0, D_MODEL, 2, dtype=F32) / D_MODEL))
    sin_inp = np.outer(pos_seq, inv_freq).astype(F32)
    return np.concatenate([np.sin(sin_inp), np.cos(sin_inp)], -1).astype(F32)


def _prep_inputs(data, memory, emb, Wq, Wkv, Wr, Wo, ffW1, ffb1, ffW2, ffb2,
                 ln1_g, ln1_b, ln2_g, ln2_b, bias_w, bias_r):
    # honor a reduced layer count (debugging)
    memory, Wq, Wkv, Wr, Wo = memory[:L], Wq[:L], Wkv[:L], Wr[:L], Wo[:L]
    ffW1, ffb1, ffW2, ffb2 = ffW1[:L], ffb1[:L], ffW2[:L], ffb2[:L]
    ln1_g, ln1_b, ln2_g, ln2_b = ln1_g[:L], ln1_b[:L], ln2_g[:L], ln2_b[:L]
    pos = _make_pos()                                  # [KLEN, D_MODEL]
    rk = np.einsum("kd,ldn->lkn", pos, Wr.astype(F32))  # [L, KLEN, 2*NDH]
    embT = np.ascontiguousarray(emb.T).astype(BF16)    # [D_MODEL, VOCAB]
    bwf = bias_w.reshape(-1).astype(F32)
    brf = bias_r.reshape(-1).astype(F32)

    def chunk(w, c):
        # [L, D, N] -> [L, 128, c, N] with row index = 128*ci + p
        L_, D_, N_ = w.shape
        return np.ascontiguousarray(
            w.reshape(L_, c, 128, N_).transpose(0, 2, 1, 3)).astype(BF16)

    def percol(v):
        # [L, D] -> [L, 128, D//128] per-partition layout
        return np.ascontiguousarray(
            v.reshape(L, -1, 128).transpose(0, 2, 1)).astype(F32)

    in_maps = []
    for c in range(NCORES):
        b, h = c // 2, c % 2
        nds = slice(NDH * h, NDH * h + NDH)
        dis = slice(DIH * h, DIH * h + DIH)
        rkTh = np.ascontiguousarray(
            rk[:, :, nds].transpose(0, 2, 1).reshape(L, 4, 128, KLEN)
        ).astype(BF16)
        memTb = np.ascontiguousarray(memory[:, b].transpose(0, 2, 1))  # [L,1024,512]
        embTh = embT[:, VSH * h : VSH * h + VSH]                       # [1024, VSH]
        embT4 = np.ascontiguousarray(
            embTh.reshape(8, 128, NVT, VT).transpose(2, 1, 0, 3))      # [NVT,128,8,VT]
        x0 = emb[np.asarray(data[b])].astype(F32)                      # [512, 1024]
        x0T = np.ascontiguousarray(
            x0.T.reshape(8, 128, QLEN).transpose(1, 0, 2)).astype(BF16)
        w1h = ffW1[:, :, dis]                                          # [L, 1024, 2048]
        w1g = np.ascontiguousarray(
            w1h.reshape(L, 8, 128, 16, 128).transpose(0, 3, 2, 1, 4)).astype(BF16)
        w2h = ffW2[:, dis, :]                                          # [L, 2048, 1024]
        w2g = np.ascontiguousarray(
            w2h.reshape(L, 16, 128, 8, 128).transpose(0, 3, 2, 1, 4)).astype(BF16)
        in_maps.append({
            "x0": x0T,
            "memT": chunk(memTb, 8),
            "wq": chunk(Wq[:, :, nds], 8),
            "wk": chunk(Wkv[:, :, nds], 8),
            "wv": chunk(Wkv[:, :, D_MODEL + NDH * h : D_MODEL + NDH * h + NDH], 8),
            "rkT": rkTh,
            "wo": chunk(Wo[:, nds, :], 4),
            "w1": w1g,
            "w2": w2g,
            "b1": np.ascontiguousarray(
                ffb1[:, dis].reshape(L, 16, 128).transpose(0, 2, 1)).astype(F32),
            "lns": np.ascontiguousarray(np.stack(
                [percol(np.asarray(v)) for v in
                 (ln1_g, ln1_b, ln2_g, ln2_b, ffb2)], axis=2)),
            "bw": np.ascontiguousarray(bwf[nds].reshape(4, 128).T),
            "br": np.ascontiguousarray(brf[nds].reshape(4, 128).T),
            "embT": embT4,
        })
    return in_maps


def _combine(results, target, emb):
    nll = np.zeros((BSZ, QLEN), dtype=np.float64)
    for b in range(BSZ):
        r0, r1 = results[2 * b], results[2 * b + 1]
        lm = np.concatenate([r0["lmax"], r1["lmax"]], axis=-1).astype(np.float64)
        ls = np.concatenate([r0["lsum"], r1["lsum"]], axis=-1).astype(np.float64)
        M = lm.max(-1)                                   # [128, 4]
        Z = (ls * np.exp(lm - M[..., None])).sum(-1)     # [128, 4]
        logZ = (M + np.log(Z)).transpose(1, 0).reshape(QLEN)  # i = 128*qc + p
        # xout[p, dc, i] = x[i, 128 dc + p]
        xf = r0["xout"].astype(np.float64).transpose(2, 1, 0).reshape(QLEN, D_MODEL)
        et = emb[np.asarray(target[b])].astype(BF16).astype(np.float64)
        tgt = (xf * et).sum(-1)
        nll[b] = logZ - tgt
    return nll.astype(F32).reshape(-1).reshape(QLEN, BSZ)


def _prep_all(inputs):
    return _prep_inputs(
        np.asarray(inputs["data"]), np.asarray(inputs["memory"], dtype=F32),
        np.asarray(inputs["emb"], dtype=F32),
        np.asarray(inputs["Wq"], dtype=F32), np.asarray(inputs["Wkv"], dtype=F32),
        np.asarray(inputs["Wr"], dtype=F32), np.asarray(inputs["Wo"], dtype=F32),
        np.asarray(inputs["ffW1"], dtype=F32), np.asarray(inputs["ffb1"], dtype=F32),
        np.asarray(inputs["ffW2"], dtype=F32), np.asarray(inputs["ffb2"], dtype=F32),
        np.asarray(inputs["ln1_g"], dtype=F32), np.asarray(inputs["ln1_b"], dtype=F32),
        np.asarray(inputs["ln2_g"], dtype=F32), np.asarray(inputs["ln2_b"], dtype=F32),
        np.asarray(inputs["bias_w"], dtype=F32), np.asarray(inputs["bias_r"], dtype=F32),
    )


def kernel(**inputs):
    nc = _get_nc()
    target = np.asarray(inputs["target"])
    emb = np.asarray(inputs["emb"], dtype=F32)
    in_maps = _prep_all(inputs)
    res = run_bass_kernel_spmd(nc, in_maps, core_ids=list(range(NCORES)))
    return _combine(res.results, target, emb)
